# revision 66
# baseline (speedup 1.0000x reference)
"""AttnGCN layer on 8 TRN2 NeuronCores — data-parallel over batch.

Per-core (one sample b):
  q = x @ Wq + bq ; k = x @ Wk + bk            (bf16 PE matmuls)
  sT[i,o] = k_i . q_o  + C'*eT[i,o]            (scores transposed; mask folded
                                                into PSUM via lhsT=e-block
                                                matmuls against a scaled
                                                identity — transposes e free)
  pT = exp(alpha*sT - C)                        (ACT, masked entries -> ~0)
  S[o] = sum_i pT[i,o]                          (ones-vector PE matmul)
  ctxT[e,o] = sum_i x16[i,e] * pT[i,o]          (PE, accumulated over i-blocks)
  ctxN = ctxT * (1/S broadcast)                 (PE broadcast + DVE mul)
  out_pre[o,c] = sum_e ctxN[e,o] * Wc[e,c]      (PE)
  x = x + out_pre ; LayerNorm(x)*gamma + beta   (DVE bn_stats/bn_aggr epilogue)

Self-contained: hardcodes shapes from the problem spec.
"""

import math
from contextlib import ExitStack

import numpy as np

import concourse.bass as bass
import concourse.tile as tile
from concourse import mybir
from concourse.vector_clock import ScopedClock

F32 = mybir.dt.float32
F32R = mybir.dt.float32r
BF16 = mybir.dt.bfloat16
FP8 = mybir.dt.float8e4

B = 8
N = 2048
D = 512
P = 128
NB = N // P       # 16 i-blocks
EC = D // P       # 4 chunks of the embed/dff dim
OC = N // 512     # 4 o-chunks of 512 attn rows
ALPHA = 1.0 / math.sqrt(D)
CPRIME = 1024.0           # mask scale inside PSUM (exactly representable)
SHIFT = 3.0               # softmax-invariant shift keeping exp() in fp8 range
CBIAS = CPRIME * ALPHA + SHIFT  # subtracted in the exp bias


# ---------------------------------------------------------------------------
# Workaround: walrus CoreV3 rejects >2 sem waits on the TileContext final
# drain ("Too many sync wait commands"). Hoist waits onto preceding nops.
def _patched_drain_and_barrier(self, tick_clock, wait_clock):
    nc = self.nc
    carrier = nc.sync.nop(nofuse=True)
    wait_clock.add_sem_waits(carrier.ins, ScopedClock({None: tick_clock.global_clock}))
    si = carrier.ins.sync_info
    waits = list(si.on_wait) if si and si.on_wait else []
    if len(waits) > 1:
        si.on_wait = waits[:1]
        for w in waits[1:]:
            n2 = nc.sync.nop(nofuse=True)
            n2.ins.sync_info = mybir.SyncInfo(on_wait=[w], on_update=[])
    nc.sync.drain()
    nc.all_engine_barrier()
    assert self.sems is not None
    popped = nc._tile_sem_poison_stack.pop()
    assert popped is self._sem_poison
    nc.clear_and_free_semaphores(list(self.sems.allocated().values()))
    nc.all_engine_barrier()


def _apply_patches():
    tile.TileContext._drain_and_barrier = _patched_drain_and_barrier


def _split_excess_waits(nc, limit=1):
    """walrus CoreV2/V3 codegen rejects instructions with >2 sem waits;
    hoist excess waits onto same-engine no-ops inserted just before."""
    n = 0
    for fn in nc.m.functions:
        for blk in fn.blocks:
            out = []
            changed = False
            for inst in blk.instructions:
                si = inst.sync_info
                waits = list(si.on_wait) if si and si.on_wait else []
                if len(waits) > limit:
                    keep = waits[-limit:]
                    for w in waits[:-limit]:
                        n += 1
                        nop = mybir.InstNoOp(name=f"I-wsplit-{n}", ins=[], outs=[])
                        nop.engine = inst.engine
                        nop.sync_info = mybir.SyncInfo(on_wait=[w], on_update=[])
                        out.append(nop)
                    si.on_wait = keep
                    changed = True
                out.append(inst)
            if changed:
                blk.instructions = out
    return n


def _r(ap):
    """View an f32 AP as float32r for full-rate PE matmuls."""
    return ap.bitcast(F32R)


def _identity(nc, ap, diag):
    nc.gpsimd.memset(ap, 0.0)
    nc.gpsimd.affine_select(
        out=ap,
        in_=ap,
        compare_op=mybir.AluOpType.not_equal,
        fill=diag,
        base=0,
        pattern=[[-1, ap.shape[0]]],
        channel_multiplier=1,
    )


def build_nc():
    nc = bass.Bass()
    x_ext = nc.declare_dram_parameter("node_fts", [N, D], F32, isOutput=False)
    e_ext = nc.declare_dram_parameter("rel_edges", [N, N], F32, isOutput=False)
    wq_ext = nc.declare_dram_parameter("Wq", [D, D], F32, isOutput=False)
    bq_ext = nc.declare_dram_parameter("bq", [D], F32, isOutput=False)
    wk_ext = nc.declare_dram_parameter("Wk", [D, D], F32, isOutput=False)
    bk_ext = nc.declare_dram_parameter("bk", [D], F32, isOutput=False)
    wc_ext = nc.declare_dram_parameter("Wc", [D, D], F32, isOutput=False)
    g_ext = nc.declare_dram_parameter("gamma", [D], F32, isOutput=False)
    be_ext = nc.declare_dram_parameter("beta", [D], F32, isOutput=False)
    out_ext = nc.declare_dram_parameter("out", [N, D], F32, isOutput=True)

    with tile.TileContext(nc) as tc, ExitStack() as ctx:
        singles = ctx.enter_context(tc.tile_pool(name="singles", bufs=1))
        wstage = ctx.enter_context(tc.tile_pool(name="wstage", bufs=2))
        xtp = ctx.enter_context(tc.tile_pool(name="xtp", bufs=2))
        eoc = ctx.enter_context(tc.tile_pool(name="eoc", bufs=6))
        e16p = ctx.enter_context(tc.tile_pool(name="e16p", bufs=4))
        ptp = ctx.enter_context(tc.tile_pool(name="ptp", bufs=4))
        ctxp = ctx.enter_context(tc.tile_pool(name="ctxp", bufs=2))
        rowp = ctx.enter_context(tc.tile_pool(name="rowp", bufs=2))
        epi = ctx.enter_context(tc.tile_pool(name="epi", bufs=2))
        xsbp = ctx.enter_context(tc.tile_pool(name="xsbp", bufs=5))
        xresp = ctx.enter_context(tc.tile_pool(name="xresp", bufs=6))
        sps = ctx.enter_context(tc.tile_pool(name="sps", bufs=3, space="PSUM"))
        ctxps_pool = ctx.enter_context(tc.tile_pool(name="ctxps", bufs=1, space="PSUM"))
        spsum = ctx.enter_context(tc.tile_pool(name="spsum", bufs=1, space="PSUM"))

        # ---- persistent tiles -------------------------------------------
        x8g = [
            singles.tile([P, 4, D], FP8, tag=f"x8g{g}", name=f"x8g{g}")
            for g in range(4)
        ]
        qt8 = singles.tile([P, EC, N], FP8, tag="qt8")
        kt8 = singles.tile([P, EC, N], FP8, tag="kt8")
        wq16 = singles.tile([P, EC, D], BF16, tag="wq16")
        wk16 = singles.tile([P, EC, D], BF16, tag="wk16")
        wc16 = singles.tile([P, EC, D], BF16, tag="wc16")
        bqt = singles.tile([P, EC], F32, tag="bqt")
        bkt = singles.tile([P, EC], F32, tag="bkt")
        gamma_b = singles.tile([P, D], F32, tag="gamma_b")
        beta_b = singles.tile([P, D], F32, tag="beta_b")
        ident32 = singles.tile([P, P], F32, tag="ident32")
        maskid16 = singles.tile([P, P], BF16, tag="maskid16")
        ones8 = singles.tile([P, 2, 16], FP8, tag="ones8")
        one32 = singles.tile([1, 1], F32, tag="one32")
        eps_t = singles.tile([P, 1], F32, tag="eps_t")
        cbias_t = singles.tile([P, 1], F32, tag="cbias_t")

        _identity(nc, ident32, 1.0)
        _identity(nc, maskid16, CPRIME)
        nc.gpsimd.memset(ones8, 1.0)
        nc.gpsimd.memset(one32, 1.0)
        nc.gpsimd.memset(eps_t, 1e-5)
        nc.gpsimd.memset(cbias_t, -CBIAS)

        # biases laid out per-partition: b[f] -> [p, fc] with f = fc*128 + p
        nc.sync.dma_start(out=bqt, in_=bq_ext[:].rearrange("(fc p) -> p fc", p=P))
        nc.sync.dma_start(out=bkt, in_=bk_ext[:].rearrange("(fc p) -> p fc", p=P))
        ge = g_ext[:]
        nc.sync.dma_start(
            out=gamma_b,
            in_=bass.AP(tensor=ge.tensor, offset=ge.offset, ap=[[0, P], *ge.ap]),
        )
        bea = be_ext[:]
        nc.sync.dma_start(
            out=beta_b,
            in_=bass.AP(tensor=bea.tensor, offset=bea.offset, ap=[[0, P], *bea.ap]),
        )

        # ---- HAM warmup: dummy matmul burst while the first DMAs land ----
        # PE clock-gate needs ~3.4us of sustained activity to go 1.2->2.4GHz;
        # burn the DMA-wait with throwaway matmuls so prep runs warm.
        warm_ps = sps.tile([P, 512], F32, tag="sps")
        for j in range(56):
            nc.tensor.matmul(
                out=warm_ps[:, (j % 4) * P : (j % 4 + 1) * P],
                lhsT=maskid16,
                rhs=maskid16,
                start=True,
                stop=True,
                skip_group_check=True,
            )

        # ---- stage q/k weights -> bf16 (ACT queue, parallel to X) -------
        # (Wc is loaded after the projections -- not needed until oc0 tail)
        for w_ext, w16 in ((wq_ext, wq16), (wk_ext, wk16)):
            ws = wstage.tile([P, EC, D], F32, tag="wstage")
            nc.scalar.dma_start(
                out=ws, in_=w_ext[:, :].rearrange("(ec p) f -> p ec f", p=P)
            )
            nc.vector.tensor_copy(out=w16, in_=ws)

        # ---- stage x + projections qT[f,i], kT[f,i] ---------------------
        xs_tiles = []
        for g in range(4):
            xs = eoc.tile([P, 4, D], F32, tag="ef")
            nc.sync.dma_start(
                out=xs,
                in_=x_ext[g * 4 * P : (g + 1) * 4 * P, :].rearrange(
                    "(ib p) e -> p ib e", p=P
                ),
            )
            xs_tiles.append(xs)
        for g in range(4):
            xs = xs_tiles[g]
            xt = xtp.tile([P, EC, 512], BF16, tag="xt")
            for ec in range(EC):
                tp = sps.tile([P, 512], F32, tag="sps")
                for k4 in range(4):
                    nc.tensor.transpose(
                        out=tp[:, k4 * P : (k4 + 1) * P],
                        in_=xs[:, k4, ec * P : (ec + 1) * P],
                        identity=ident32,
                    )
                nc.vector.tensor_copy(out=xt[:, ec, :], in_=tp)
            for w16, bt, dst in ((wq16, bqt, qt8), (wk16, bkt, kt8)):
                for fc in range(EC):
                    ps = sps.tile([P, 512], F32, tag="sps")
                    for ec in range(EC):
                        nc.tensor.matmul(
                            out=ps,
                            lhsT=w16[:, ec, fc * P : (fc + 1) * P],
                            rhs=xt[:, ec, :],
                            start=(ec == 0),
                            stop=(ec == EC - 1),
                            skip_group_check=True,
                        )
                    nc.scalar.activation(
                        out=dst[:, fc, g * 512 : (g + 1) * 512],
                        in_=ps,
                        func=mybir.ActivationFunctionType.Identity,
                        bias=bt[:, fc : fc + 1],
                        scale=1.0,
                    )

        # edge loader: one quarter of an o-chunk's mask columns at a time
        def emit_e_quarter(oc, q):
            ef = eoc.tile([P, 4, 512], F32, tag="ef", name=f"ef{oc}{q}")
            nc.gpsimd.dma_start(
                out=ef,
                in_=e_ext[
                    oc * 512 : (oc + 1) * 512, q * 512 : (q + 1) * 512
                ].rearrange("(s p) f -> p s f", p=P),
            )
            e16 = e16p.tile([P, 4, 512], BF16, tag="e16", name=f"e16{oc}{q}")
            # f32->bf16 cast split across DVE and ACT (gpsimd is byte-limited)
            if q % 2 == 0:
                nc.vector.tensor_copy(out=e16, in_=ef)
            else:
                nc.scalar.copy(out=e16, in_=ef)
            return e16

        # oc0 edges pre-issued so the gpsimd DMA ring starts before x8g casts
        e16_pre = [emit_e_quarter(0, q) for q in range(2)]

        for g in range(4):
            nc.gpsimd.tensor_copy(out=x8g[g], in_=xs_tiles[g])

        # deferred Wc staging (first used at the oc0 tail)
        ws = wstage.tile([P, EC, D], F32, tag="wstage")
        nc.scalar.dma_start(
            out=ws, in_=wc_ext[:, :].rearrange("(ec p) f -> p ec f", p=P)
        )
        nc.vector.tensor_copy(out=wc16, in_=ws)

        # ---- main loop over o-chunks ------------------------------------
        for oc in range(OC):
            # residual rows for this chunk (prefetch early)
            xres_tiles = []
            for os4 in range(4):
                r0 = (oc * 4 + os4) * P
                xr = xresp.tile([P, D], F32, tag="xres")
                nc.sync.dma_start(out=xr, in_=x_ext[r0 : r0 + P, :])
                xres_tiles.append(xr)

            if oc == 0:
                e16_q = e16_pre + [emit_e_quarter(0, q) for q in range(2, 4)]
            else:
                e16_q = [emit_e_quarter(oc, q) for q in range(4)]

            ctx_ps = ctxps_pool.tile([P, EC, 512], F32, tag="ctxps")
            s_ps = spsum.tile([1, 512], F32, tag="spsum")

            pt2 = None
            for ib in range(NB):
                e16, il = e16_q[ib // 4], ib % 4
                sp = sps.tile([P, 512], F32, tag="sps")
                for s in range(4):
                    # start=True clears the whole PSUM bank -> only on s==0;
                    # later mask MMs hit has_written=0 and write directly.
                    nc.tensor.matmul(
                        out=sp[:, s * P : (s + 1) * P],
                        lhsT=e16[:, s, il * P : (il + 1) * P],
                        rhs=maskid16,
                        start=(s == 0),
                        stop=False,
                        skip_group_check=True,
                    )
                for dc in (0, 2):
                    nc.tensor.matmul(
                        out=sp,
                        lhsT=kt8[:, dc : dc + 2, ib * P : (ib + 1) * P],
                        rhs=qt8[:, dc : dc + 2, oc * 512 : (oc + 1) * 512],
                        start=False,
                        stop=(dc == 2),
                        perf_mode=mybir.MatmulPerfMode.DoubleRow,
                        skip_group_check=True,
                    )
                if ib % 2 == 0:
                    pt2 = ptp.tile([P, 2, 512], FP8, tag="pt")
                nc.scalar.activation(
                    out=pt2[:, ib % 2, :],
                    in_=sp,
                    func=mybir.ActivationFunctionType.Exp,
                    bias=cbias_t[:, 0:1],
                    scale=ALPHA,
                )
                if ib % 2 == 1:
                    j = (ib % 4) - 1
                    for ec in range(EC):
                        nc.tensor.matmul(
                            out=ctx_ps[:, ec, :],
                            lhsT=x8g[ib // 4][:, j : j + 2, ec * P : (ec + 1) * P],
                            rhs=pt2,
                            start=(ib == 1),
                            stop=(ib == NB - 1),
                            perf_mode=mybir.MatmulPerfMode.DoubleRow,
                            skip_group_check=True,
                        )
                    nc.tensor.matmul(
                        out=s_ps,
                        lhsT=ones8[:, :, 0:1],
                        rhs=pt2,
                        start=(ib == 1),
                        stop=(ib == NB - 1),
                        perf_mode=mybir.MatmulPerfMode.DoubleRow,
                        skip_group_check=True,
                    )

            # unnormalized ctx -> SBUF bf16 (independent of S: overlaps)
            ctx16 = ctxp.tile([P, EC, 512], BF16, tag="ctx16")
            for ec in range(EC):
                nc.scalar.copy(out=ctx16[:, ec, :], in_=ctx_ps[:, ec, :])

            # 1/S as a per-partition column: S row -> PE transpose -> recip
            s_sb = rowp.tile([1, 512], F32, tag="s_sb")
            nc.vector.tensor_copy(out=s_sb, in_=s_ps)
            s_col = sps.tile([P, 4], F32, tag="sps")
            for os4 in range(4):
                nc.tensor.matmul(
                    out=s_col[:, os4 : os4 + 1],
                    lhsT=s_sb[0:1, os4 * P : (os4 + 1) * P],
                    rhs=one32,
                    is_transpose=True,
                    start=(os4 == 0),
                    stop=(os4 == 3),
                    skip_group_check=True,
                )
            rs_col = rowp.tile([P, 4], F32, tag="rs_col")
            nc.vector.reciprocal(out=rs_col, in_=s_col)

            # out_pre = ctx_unnorm @ Wc ; scale rows by 1/S ; residual + LN
            # pass 1: x = out_pre/S + xres, batched mean/var
            x_tiles = []
            mv4 = epi.tile([P, 4, 2], F32, tag="mv4")
            for os4 in range(4):
                opre = sps.tile([P, 512], F32, tag="sps")
                for ec in range(EC):
                    nc.tensor.matmul(
                        out=opre,
                        lhsT=ctx16[:, ec, os4 * P : (os4 + 1) * P],
                        rhs=wc16[:, ec, :],
                        start=(ec == 0),
                        stop=(ec == EC - 1),
                        skip_group_check=True,
                    )
                t0 = epi.tile([P, D], F32, tag="t0")
                nc.vector.tensor_scalar_mul(t0, opre, rs_col[:, os4 : os4 + 1])
                x_sb = xsbp.tile([P, D], F32, tag="x_sb")
                nc.vector.tensor_add(x_sb, t0, xres_tiles[os4])
                x_tiles.append(x_sb)
                stats = epi.tile([P, 6], F32, tag="stats")
                nc.vector.bn_stats(out=stats, in_=x_sb)
                nc.vector.bn_aggr(out=mv4[:, os4, :], in_=stats)
            # one Sqrt + one reciprocal per o-chunk (avoids ACT table thrash)
            sd4 = epi.tile([P, 4], F32, tag="sd4")
            nc.scalar.activation(
                out=sd4,
                in_=mv4[:, :, 1],
                func=mybir.ActivationFunctionType.Sqrt,
                bias=eps_t[:, 0:1],
                scale=1.0,
            )
            rs4 = epi.tile([P, 4], F32, tag="rs4")
            nc.vector.reciprocal(out=rs4, in_=sd4)
            # pass 2: normalize, gamma/beta, store
            for os4 in range(4):
                t_sb = epi.tile([P, D], F32, tag="t_sb")
                nc.vector.tensor_scalar(
                    t_sb,
                    x_tiles[os4],
                    mv4[:, os4, 0:1],
                    rs4[:, os4 : os4 + 1],
                    mybir.AluOpType.subtract,
                    mybir.AluOpType.mult,
                )
                g_sb = epi.tile([P, D], F32, tag="g_sb")
                nc.gpsimd.tensor_mul(g_sb, t_sb, gamma_b)
                o_sb = epi.tile([P, D], F32, tag="o_sb")
                nc.vector.tensor_add(o_sb, g_sb, beta_b)
                r0 = (oc * 4 + os4) * P
                nc.sync.dma_start(out=out_ext[r0 : r0 + P, :], in_=o_sb)

    _split_excess_waits(nc)
    return nc


_NC_CACHE = None


def kernel(**inputs) -> np.ndarray:
    global _NC_CACHE
    _apply_patches()
    from concourse.bass_utils import run_bass_kernel_spmd

    node_fts = np.ascontiguousarray(np.asarray(inputs["node_fts"], dtype=np.float32))
    rel_edges = np.ascontiguousarray(np.asarray(inputs["rel_edges"], dtype=np.float32))
    shared = {
        k: np.ascontiguousarray(np.asarray(inputs[k], dtype=np.float32))
        for k in ("Wq", "bq", "Wk", "bk", "Wc", "gamma", "beta")
    }
    if _NC_CACHE is None:
        _NC_CACHE = build_nc()
    in_maps = [
        {"node_fts": node_fts[b], "rel_edges": rel_edges[b], **shared}
        for b in range(B)
    ]
    res = run_bass_kernel_spmd(_NC_CACHE, in_maps, core_ids=list(range(B)))
    return np.stack([res.results[b]["out"] for b in range(B)]).astype(np.float32)


# revision 67
# speedup vs baseline: 1.0109x; 1.0109x over previous
"""AttnGCN layer on 8 TRN2 NeuronCores — data-parallel over batch.

Per-core (one sample b):
  q = x @ Wq + bq ; k = x @ Wk + bk            (bf16 PE matmuls)
  sT[i,o] = k_i . q_o  + C'*eT[i,o]            (scores transposed; mask folded
                                                into PSUM via lhsT=e-block
                                                matmuls against a scaled
                                                identity — transposes e free)
  pT = exp(alpha*sT - C)                        (ACT, masked entries -> ~0)
  S[o] = sum_i pT[i,o]                          (ones-vector PE matmul)
  ctxT[e,o] = sum_i x16[i,e] * pT[i,o]          (PE, accumulated over i-blocks)
  ctxN = ctxT * (1/S broadcast)                 (PE broadcast + DVE mul)
  out_pre[o,c] = sum_e ctxN[e,o] * Wc[e,c]      (PE)
  x = x + out_pre ; LayerNorm(x)*gamma + beta   (DVE bn_stats/bn_aggr epilogue)

Self-contained: hardcodes shapes from the problem spec.
"""

import math
from contextlib import ExitStack

import numpy as np

import concourse.bass as bass
import concourse.tile as tile
from concourse import mybir
from concourse.vector_clock import ScopedClock

F32 = mybir.dt.float32
F32R = mybir.dt.float32r
BF16 = mybir.dt.bfloat16
FP8 = mybir.dt.float8e4

B = 8
N = 2048
D = 512
P = 128
NB = N // P       # 16 i-blocks
EC = D // P       # 4 chunks of the embed/dff dim
OC = N // 512     # 4 o-chunks of 512 attn rows
ALPHA = 1.0 / math.sqrt(D)
CPRIME = 1024.0           # mask scale inside PSUM (exactly representable)
SHIFT = 3.0               # softmax-invariant shift keeping exp() in fp8 range
CBIAS = CPRIME * ALPHA + SHIFT  # subtracted in the exp bias


# ---------------------------------------------------------------------------
# Workaround: walrus CoreV3 rejects >2 sem waits on the TileContext final
# drain ("Too many sync wait commands"). Hoist waits onto preceding nops.
def _patched_drain_and_barrier(self, tick_clock, wait_clock):
    nc = self.nc
    carrier = nc.sync.nop(nofuse=True)
    wait_clock.add_sem_waits(carrier.ins, ScopedClock({None: tick_clock.global_clock}))
    si = carrier.ins.sync_info
    waits = list(si.on_wait) if si and si.on_wait else []
    if len(waits) > 1:
        si.on_wait = waits[:1]
        for w in waits[1:]:
            n2 = nc.sync.nop(nofuse=True)
            n2.ins.sync_info = mybir.SyncInfo(on_wait=[w], on_update=[])
    nc.sync.drain()
    nc.all_engine_barrier()
    assert self.sems is not None
    popped = nc._tile_sem_poison_stack.pop()
    assert popped is self._sem_poison
    nc.clear_and_free_semaphores(list(self.sems.allocated().values()))
    nc.all_engine_barrier()


def _apply_patches():
    tile.TileContext._drain_and_barrier = _patched_drain_and_barrier


def _split_excess_waits(nc, limit=1):
    """walrus CoreV2/V3 codegen rejects instructions with >2 sem waits;
    hoist excess waits onto same-engine no-ops inserted just before."""
    n = 0
    for fn in nc.m.functions:
        for blk in fn.blocks:
            out = []
            changed = False
            for inst in blk.instructions:
                si = inst.sync_info
                waits = list(si.on_wait) if si and si.on_wait else []
                if len(waits) > limit:
                    keep = waits[-limit:]
                    for w in waits[:-limit]:
                        n += 1
                        nop = mybir.InstNoOp(name=f"I-wsplit-{n}", ins=[], outs=[])
                        nop.engine = inst.engine
                        nop.sync_info = mybir.SyncInfo(on_wait=[w], on_update=[])
                        out.append(nop)
                    si.on_wait = keep
                    changed = True
                out.append(inst)
            if changed:
                blk.instructions = out
    return n


def _r(ap):
    """View an f32 AP as float32r for full-rate PE matmuls."""
    return ap.bitcast(F32R)


def _identity(nc, ap, diag):
    nc.gpsimd.memset(ap, 0.0)
    nc.gpsimd.affine_select(
        out=ap,
        in_=ap,
        compare_op=mybir.AluOpType.not_equal,
        fill=diag,
        base=0,
        pattern=[[-1, ap.shape[0]]],
        channel_multiplier=1,
    )


def build_nc():
    nc = bass.Bass()
    x_ext = nc.declare_dram_parameter("node_fts", [N, D], F32, isOutput=False)
    e_ext = nc.declare_dram_parameter("rel_edges", [N, N], F32, isOutput=False)
    wq_ext = nc.declare_dram_parameter("Wq", [D, D], F32, isOutput=False)
    bq_ext = nc.declare_dram_parameter("bq", [D], F32, isOutput=False)
    wk_ext = nc.declare_dram_parameter("Wk", [D, D], F32, isOutput=False)
    bk_ext = nc.declare_dram_parameter("bk", [D], F32, isOutput=False)
    wc_ext = nc.declare_dram_parameter("Wc", [D, D], F32, isOutput=False)
    g_ext = nc.declare_dram_parameter("gamma", [D], F32, isOutput=False)
    be_ext = nc.declare_dram_parameter("beta", [D], F32, isOutput=False)
    out_ext = nc.declare_dram_parameter("out", [N, D], F32, isOutput=True)

    with tile.TileContext(nc) as tc, ExitStack() as ctx:
        singles = ctx.enter_context(tc.tile_pool(name="singles", bufs=1))
        wstage = ctx.enter_context(tc.tile_pool(name="wstage", bufs=2))
        xtp = ctx.enter_context(tc.tile_pool(name="xtp", bufs=2))
        eoc = ctx.enter_context(tc.tile_pool(name="eoc", bufs=6))
        e16p = ctx.enter_context(tc.tile_pool(name="e16p", bufs=4))
        ptp = ctx.enter_context(tc.tile_pool(name="ptp", bufs=4))
        ctxp = ctx.enter_context(tc.tile_pool(name="ctxp", bufs=2))
        rowp = ctx.enter_context(tc.tile_pool(name="rowp", bufs=2))
        epi = ctx.enter_context(tc.tile_pool(name="epi", bufs=2))
        xsbp = ctx.enter_context(tc.tile_pool(name="xsbp", bufs=5))
        xresp = ctx.enter_context(tc.tile_pool(name="xresp", bufs=6))
        sps = ctx.enter_context(tc.tile_pool(name="sps", bufs=3, space="PSUM"))
        ctxps_pool = ctx.enter_context(tc.tile_pool(name="ctxps", bufs=1, space="PSUM"))
        spsum = ctx.enter_context(tc.tile_pool(name="spsum", bufs=1, space="PSUM"))

        # ---- persistent tiles -------------------------------------------
        x8g = [
            singles.tile([P, 4, D], FP8, tag=f"x8g{g}", name=f"x8g{g}")
            for g in range(4)
        ]
        qt8 = singles.tile([P, EC, N], FP8, tag="qt8")
        kt8 = singles.tile([P, EC, N], FP8, tag="kt8")
        wq16 = singles.tile([P, EC, D], BF16, tag="wq16")
        wk16 = singles.tile([P, EC, D], BF16, tag="wk16")
        wc16 = singles.tile([P, EC, D], BF16, tag="wc16")
        bqt = singles.tile([P, EC], F32, tag="bqt")
        bkt = singles.tile([P, EC], F32, tag="bkt")
        gamma_b = singles.tile([P, D], F32, tag="gamma_b")
        beta_b = singles.tile([P, D], F32, tag="beta_b")
        ident32 = singles.tile([P, P], F32, tag="ident32")
        maskid16 = singles.tile([P, P], BF16, tag="maskid16")
        ones8 = singles.tile([P, 2, 16], FP8, tag="ones8")
        one32 = singles.tile([1, 1], F32, tag="one32")
        eps_t = singles.tile([P, 1], F32, tag="eps_t")
        cbias_t = singles.tile([P, 1], F32, tag="cbias_t")

        _identity(nc, ident32, 1.0)
        _identity(nc, maskid16, CPRIME)
        nc.gpsimd.memset(ones8, 1.0)
        nc.gpsimd.memset(one32, 1.0)
        nc.gpsimd.memset(eps_t, 1e-5)
        nc.gpsimd.memset(cbias_t, -CBIAS)

        # biases laid out per-partition: b[f] -> [p, fc] with f = fc*128 + p
        nc.sync.dma_start(out=bqt, in_=bq_ext[:].rearrange("(fc p) -> p fc", p=P))
        nc.sync.dma_start(out=bkt, in_=bk_ext[:].rearrange("(fc p) -> p fc", p=P))
        ge = g_ext[:]
        nc.sync.dma_start(
            out=gamma_b,
            in_=bass.AP(tensor=ge.tensor, offset=ge.offset, ap=[[0, P], *ge.ap]),
        )
        bea = be_ext[:]
        nc.sync.dma_start(
            out=beta_b,
            in_=bass.AP(tensor=bea.tensor, offset=bea.offset, ap=[[0, P], *bea.ap]),
        )

        # ---- HAM warmup: dummy matmul burst while the first DMAs land ----
        # PE clock-gate needs ~3.4us of sustained activity to go 1.2->2.4GHz;
        # burn the DMA-wait with throwaway matmuls so prep runs warm.
        warm_ps = sps.tile([P, 512], F32, tag="sps")
        for j in range(56):
            nc.tensor.matmul(
                out=warm_ps[:, (j % 4) * P : (j % 4 + 1) * P],
                lhsT=maskid16,
                rhs=maskid16,
                start=True,
                stop=True,
                skip_group_check=True,
            )

        # ---- stage q/k weights -> bf16 (ACT queue, parallel to X) -------
        # (Wc is loaded after the projections -- not needed until oc0 tail)
        for w_ext, w16 in ((wq_ext, wq16), (wk_ext, wk16)):
            ws = wstage.tile([P, EC, D], F32, tag="wstage")
            nc.scalar.dma_start(
                out=ws, in_=w_ext[:, :].rearrange("(ec p) f -> p ec f", p=P)
            )
            nc.vector.tensor_copy(out=w16, in_=ws)

        # ---- stage x + projections qT[f,i], kT[f,i] ---------------------
        xs_tiles = []
        for g in range(4):
            xs = eoc.tile([P, 4, D], F32, tag="ef")
            nc.sync.dma_start(
                out=xs,
                in_=x_ext[g * 4 * P : (g + 1) * 4 * P, :].rearrange(
                    "(ib p) e -> p ib e", p=P
                ),
            )
            xs_tiles.append(xs)
        for g in range(4):
            xs = xs_tiles[g]
            xt = xtp.tile([P, EC, 512], BF16, tag="xt")
            for ec in range(EC):
                tp = sps.tile([P, 512], F32, tag="sps")
                for k4 in range(4):
                    nc.tensor.transpose(
                        out=tp[:, k4 * P : (k4 + 1) * P],
                        in_=xs[:, k4, ec * P : (ec + 1) * P],
                        identity=ident32,
                    )
                nc.vector.tensor_copy(out=xt[:, ec, :], in_=tp)
            for w16, bt, dst in ((wq16, bqt, qt8), (wk16, bkt, kt8)):
                for fc in range(EC):
                    ps = sps.tile([P, 512], F32, tag="sps")
                    for ec in range(EC):
                        nc.tensor.matmul(
                            out=ps,
                            lhsT=w16[:, ec, fc * P : (fc + 1) * P],
                            rhs=xt[:, ec, :],
                            start=(ec == 0),
                            stop=(ec == EC - 1),
                            skip_group_check=True,
                        )
                    nc.scalar.activation(
                        out=dst[:, fc, g * 512 : (g + 1) * 512],
                        in_=ps,
                        func=mybir.ActivationFunctionType.Identity,
                        bias=bt[:, fc : fc + 1],
                        scale=1.0,
                    )

        # edge loader: one quarter of an o-chunk's mask columns at a time
        def emit_e_quarter(oc, q):
            ef = eoc.tile([P, 4, 512], F32, tag="ef", name=f"ef{oc}{q}")
            nc.sync.dma_start(
                out=ef,
                in_=e_ext[
                    oc * 512 : (oc + 1) * 512, q * 512 : (q + 1) * 512
                ].rearrange("(s p) f -> p s f", p=P),
            )
            e16 = e16p.tile([P, 4, 512], BF16, tag="e16", name=f"e16{oc}{q}")
            # f32->bf16 cast split across DVE and ACT (gpsimd is byte-limited)
            if q % 2 == 0:
                nc.vector.tensor_copy(out=e16, in_=ef)
            else:
                nc.scalar.copy(out=e16, in_=ef)
            return e16

        # oc0 edges pre-issued so the gpsimd DMA ring starts before x8g casts
        e16_pre = [emit_e_quarter(0, q) for q in range(2)]

        for g in range(4):
            nc.gpsimd.tensor_copy(out=x8g[g], in_=xs_tiles[g])

        # deferred Wc staging (first used at the oc0 tail)
        ws = wstage.tile([P, EC, D], F32, tag="wstage")
        nc.scalar.dma_start(
            out=ws, in_=wc_ext[:, :].rearrange("(ec p) f -> p ec f", p=P)
        )
        nc.vector.tensor_copy(out=wc16, in_=ws)

        # ---- main loop over o-chunks ------------------------------------
        for oc in range(OC):
            # residual rows for this chunk (prefetch early)
            xres_tiles = []
            for os4 in range(4):
                r0 = (oc * 4 + os4) * P
                xr = xresp.tile([P, D], F32, tag="xres")
                nc.sync.dma_start(out=xr, in_=x_ext[r0 : r0 + P, :])
                xres_tiles.append(xr)

            if oc == 0:
                e16_q = e16_pre + [emit_e_quarter(0, q) for q in range(2, 4)]
            else:
                e16_q = [emit_e_quarter(oc, q) for q in range(4)]

            ctx_ps = ctxps_pool.tile([P, EC, 512], F32, tag="ctxps")
            s_ps = spsum.tile([1, 512], F32, tag="spsum")

            pt2 = None
            for ib in range(NB):
                e16, il = e16_q[ib // 4], ib % 4
                sp = sps.tile([P, 512], F32, tag="sps")
                for s in range(4):
                    # start=True clears the whole PSUM bank -> only on s==0;
                    # later mask MMs hit has_written=0 and write directly.
                    nc.tensor.matmul(
                        out=sp[:, s * P : (s + 1) * P],
                        lhsT=e16[:, s, il * P : (il + 1) * P],
                        rhs=maskid16,
                        start=(s == 0),
                        stop=False,
                        skip_group_check=True,
                    )
                for dc in (0, 2):
                    nc.tensor.matmul(
                        out=sp,
                        lhsT=kt8[:, dc : dc + 2, ib * P : (ib + 1) * P],
                        rhs=qt8[:, dc : dc + 2, oc * 512 : (oc + 1) * 512],
                        start=False,
                        stop=(dc == 2),
                        perf_mode=mybir.MatmulPerfMode.DoubleRow,
                        skip_group_check=True,
                    )
                if ib % 2 == 0:
                    pt2 = ptp.tile([P, 2, 512], FP8, tag="pt")
                nc.scalar.activation(
                    out=pt2[:, ib % 2, :],
                    in_=sp,
                    func=mybir.ActivationFunctionType.Exp,
                    bias=cbias_t[:, 0:1],
                    scale=ALPHA,
                )
                if ib % 2 == 1:
                    j = (ib % 4) - 1
                    for ec in range(EC):
                        nc.tensor.matmul(
                            out=ctx_ps[:, ec, :],
                            lhsT=x8g[ib // 4][:, j : j + 2, ec * P : (ec + 1) * P],
                            rhs=pt2,
                            start=(ib == 1),
                            stop=(ib == NB - 1),
                            perf_mode=mybir.MatmulPerfMode.DoubleRow,
                            skip_group_check=True,
                        )
                    nc.tensor.matmul(
                        out=s_ps,
                        lhsT=ones8[:, :, 0:1],
                        rhs=pt2,
                        start=(ib == 1),
                        stop=(ib == NB - 1),
                        perf_mode=mybir.MatmulPerfMode.DoubleRow,
                        skip_group_check=True,
                    )

            # unnormalized ctx -> SBUF bf16 (independent of S: overlaps)
            ctx16 = ctxp.tile([P, EC, 512], BF16, tag="ctx16")
            for ec in range(EC):
                nc.scalar.copy(out=ctx16[:, ec, :], in_=ctx_ps[:, ec, :])

            # 1/S as a per-partition column: S row -> PE transpose -> recip
            s_sb = rowp.tile([1, 512], F32, tag="s_sb")
            nc.vector.tensor_copy(out=s_sb, in_=s_ps)
            s_col = sps.tile([P, 4], F32, tag="sps")
            for os4 in range(4):
                nc.tensor.matmul(
                    out=s_col[:, os4 : os4 + 1],
                    lhsT=s_sb[0:1, os4 * P : (os4 + 1) * P],
                    rhs=one32,
                    is_transpose=True,
                    start=(os4 == 0),
                    stop=(os4 == 3),
                    skip_group_check=True,
                )
            rs_col = rowp.tile([P, 4], F32, tag="rs_col")
            nc.vector.reciprocal(out=rs_col, in_=s_col)

            # out_pre = ctx_unnorm @ Wc ; scale rows by 1/S ; residual + LN
            # pass 1: x = out_pre/S + xres, batched mean/var
            x_tiles = []
            mv4 = epi.tile([P, 4, 2], F32, tag="mv4")
            for os4 in range(4):
                opre = sps.tile([P, 512], F32, tag="sps")
                for ec in range(EC):
                    nc.tensor.matmul(
                        out=opre,
                        lhsT=ctx16[:, ec, os4 * P : (os4 + 1) * P],
                        rhs=wc16[:, ec, :],
                        start=(ec == 0),
                        stop=(ec == EC - 1),
                        skip_group_check=True,
                    )
                t0 = epi.tile([P, D], F32, tag="t0")
                nc.vector.tensor_scalar_mul(t0, opre, rs_col[:, os4 : os4 + 1])
                x_sb = xsbp.tile([P, D], F32, tag="x_sb")
                nc.vector.tensor_add(x_sb, t0, xres_tiles[os4])
                x_tiles.append(x_sb)
                stats = epi.tile([P, 6], F32, tag="stats")
                nc.vector.bn_stats(out=stats, in_=x_sb)
                nc.vector.bn_aggr(out=mv4[:, os4, :], in_=stats)
            # one Sqrt + one reciprocal per o-chunk (avoids ACT table thrash)
            sd4 = epi.tile([P, 4], F32, tag="sd4")
            nc.scalar.activation(
                out=sd4,
                in_=mv4[:, :, 1],
                func=mybir.ActivationFunctionType.Sqrt,
                bias=eps_t[:, 0:1],
                scale=1.0,
            )
            rs4 = epi.tile([P, 4], F32, tag="rs4")
            nc.vector.reciprocal(out=rs4, in_=sd4)
            # pass 2: normalize, gamma/beta, store
            for os4 in range(4):
                t_sb = epi.tile([P, D], F32, tag="t_sb")
                nc.vector.tensor_scalar(
                    t_sb,
                    x_tiles[os4],
                    mv4[:, os4, 0:1],
                    rs4[:, os4 : os4 + 1],
                    mybir.AluOpType.subtract,
                    mybir.AluOpType.mult,
                )
                g_sb = epi.tile([P, D], F32, tag="g_sb")
                nc.gpsimd.tensor_mul(g_sb, t_sb, gamma_b)
                o_sb = epi.tile([P, D], F32, tag="o_sb")
                nc.vector.tensor_add(o_sb, g_sb, beta_b)
                r0 = (oc * 4 + os4) * P
                nc.sync.dma_start(out=out_ext[r0 : r0 + P, :], in_=o_sb)

    _split_excess_waits(nc)
    return nc


_NC_CACHE = None


def kernel(**inputs) -> np.ndarray:
    global _NC_CACHE
    _apply_patches()
    from concourse.bass_utils import run_bass_kernel_spmd

    node_fts = np.ascontiguousarray(np.asarray(inputs["node_fts"], dtype=np.float32))
    rel_edges = np.ascontiguousarray(np.asarray(inputs["rel_edges"], dtype=np.float32))
    shared = {
        k: np.ascontiguousarray(np.asarray(inputs[k], dtype=np.float32))
        for k in ("Wq", "bq", "Wk", "bk", "Wc", "gamma", "beta")
    }
    if _NC_CACHE is None:
        _NC_CACHE = build_nc()
    in_maps = [
        {"node_fts": node_fts[b], "rel_edges": rel_edges[b], **shared}
        for b in range(B)
    ]
    res = run_bass_kernel_spmd(_NC_CACHE, in_maps, core_ids=list(range(B)))
    return np.stack([res.results[b]["out"] for b in range(B)]).astype(np.float32)


# revision 68
# speedup vs baseline: 1.0661x; 1.0546x over previous
"""AttnGCN layer on 8 TRN2 NeuronCores — data-parallel over batch.

Per-core (one sample b):
  q = x @ Wq + bq ; k = x @ Wk + bk            (bf16 PE matmuls)
  sT[i,o] = k_i . q_o  + C'*eT[i,o]            (scores transposed; mask folded
                                                into PSUM via lhsT=e-block
                                                matmuls against a scaled
                                                identity — transposes e free)
  pT = exp(alpha*sT - C)                        (ACT, masked entries -> ~0)
  S[o] = sum_i pT[i,o]                          (ones-vector PE matmul)
  ctxT[e,o] = sum_i x16[i,e] * pT[i,o]          (PE, accumulated over i-blocks)
  ctxN = ctxT * (1/S broadcast)                 (PE broadcast + DVE mul)
  out_pre[o,c] = sum_e ctxN[e,o] * Wc[e,c]      (PE)
  x = x + out_pre ; LayerNorm(x)*gamma + beta   (DVE bn_stats/bn_aggr epilogue)

Self-contained: hardcodes shapes from the problem spec.
"""

import math
from contextlib import ExitStack

import numpy as np

import concourse.bass as bass
import concourse.tile as tile
from concourse import mybir
from concourse.vector_clock import ScopedClock

F32 = mybir.dt.float32
F32R = mybir.dt.float32r
BF16 = mybir.dt.bfloat16
FP8 = mybir.dt.float8e4

B = 8
N = 2048
D = 512
P = 128
NB = N // P       # 16 i-blocks
EC = D // P       # 4 chunks of the embed/dff dim
OC = N // 512     # 4 o-chunks of 512 attn rows
ALPHA = 1.0 / math.sqrt(D)
CPRIME = 1024.0           # mask scale inside PSUM (exactly representable)
SHIFT = 3.0               # softmax-invariant shift keeping exp() in fp8 range
CBIAS = CPRIME * ALPHA + SHIFT  # subtracted in the exp bias


# ---------------------------------------------------------------------------
# Workaround: walrus CoreV3 rejects >2 sem waits on the TileContext final
# drain ("Too many sync wait commands"). Hoist waits onto preceding nops.
def _patched_drain_and_barrier(self, tick_clock, wait_clock):
    nc = self.nc
    carrier = nc.sync.nop(nofuse=True)
    wait_clock.add_sem_waits(carrier.ins, ScopedClock({None: tick_clock.global_clock}))
    si = carrier.ins.sync_info
    waits = list(si.on_wait) if si and si.on_wait else []
    if len(waits) > 1:
        si.on_wait = waits[:1]
        for w in waits[1:]:
            n2 = nc.sync.nop(nofuse=True)
            n2.ins.sync_info = mybir.SyncInfo(on_wait=[w], on_update=[])
    nc.sync.drain()
    nc.all_engine_barrier()
    assert self.sems is not None
    popped = nc._tile_sem_poison_stack.pop()
    assert popped is self._sem_poison
    nc.clear_and_free_semaphores(list(self.sems.allocated().values()))
    nc.all_engine_barrier()


def _apply_patches():
    tile.TileContext._drain_and_barrier = _patched_drain_and_barrier


def _split_excess_waits(nc, limit=1):
    """walrus CoreV2/V3 codegen rejects instructions with >2 sem waits;
    hoist excess waits onto same-engine no-ops inserted just before."""
    n = 0
    for fn in nc.m.functions:
        for blk in fn.blocks:
            out = []
            changed = False
            for inst in blk.instructions:
                si = inst.sync_info
                waits = list(si.on_wait) if si and si.on_wait else []
                if len(waits) > limit:
                    keep = waits[-limit:]
                    for w in waits[:-limit]:
                        n += 1
                        nop = mybir.InstNoOp(name=f"I-wsplit-{n}", ins=[], outs=[])
                        nop.engine = inst.engine
                        nop.sync_info = mybir.SyncInfo(on_wait=[w], on_update=[])
                        out.append(nop)
                    si.on_wait = keep
                    changed = True
                out.append(inst)
            if changed:
                blk.instructions = out
    return n


def _r(ap):
    """View an f32 AP as float32r for full-rate PE matmuls."""
    return ap.bitcast(F32R)


def _identity(nc, ap, diag):
    nc.gpsimd.memset(ap, 0.0)
    nc.gpsimd.affine_select(
        out=ap,
        in_=ap,
        compare_op=mybir.AluOpType.not_equal,
        fill=diag,
        base=0,
        pattern=[[-1, ap.shape[0]]],
        channel_multiplier=1,
    )


def build_nc():
    nc = bass.Bass()
    x_ext = nc.declare_dram_parameter("node_fts", [N, D], F32, isOutput=False)
    e_ext = nc.declare_dram_parameter("rel_edges", [N, N], F32, isOutput=False)
    wq_ext = nc.declare_dram_parameter("Wq", [D, D], F32, isOutput=False)
    bq_ext = nc.declare_dram_parameter("bq", [D], F32, isOutput=False)
    wk_ext = nc.declare_dram_parameter("Wk", [D, D], F32, isOutput=False)
    bk_ext = nc.declare_dram_parameter("bk", [D], F32, isOutput=False)
    wc_ext = nc.declare_dram_parameter("Wc", [D, D], F32, isOutput=False)
    g_ext = nc.declare_dram_parameter("gamma", [D], F32, isOutput=False)
    be_ext = nc.declare_dram_parameter("beta", [D], F32, isOutput=False)
    out_ext = nc.declare_dram_parameter("out", [N, D], F32, isOutput=True)

    with tile.TileContext(nc) as tc, ExitStack() as ctx:
        singles = ctx.enter_context(tc.tile_pool(name="singles", bufs=1))
        wstage = ctx.enter_context(tc.tile_pool(name="wstage", bufs=2))
        xtp = ctx.enter_context(tc.tile_pool(name="xtp", bufs=2))
        eoc = ctx.enter_context(tc.tile_pool(name="eoc", bufs=6))
        e16p = ctx.enter_context(tc.tile_pool(name="e16p", bufs=4))
        ptp = ctx.enter_context(tc.tile_pool(name="ptp", bufs=4))
        ctxp = ctx.enter_context(tc.tile_pool(name="ctxp", bufs=2))
        rowp = ctx.enter_context(tc.tile_pool(name="rowp", bufs=2))
        epi = ctx.enter_context(tc.tile_pool(name="epi", bufs=2))
        xsbp = ctx.enter_context(tc.tile_pool(name="xsbp", bufs=5))
        xresp = ctx.enter_context(tc.tile_pool(name="xresp", bufs=6))
        sps = ctx.enter_context(tc.tile_pool(name="sps", bufs=3, space="PSUM"))
        ctxps_pool = ctx.enter_context(tc.tile_pool(name="ctxps", bufs=1, space="PSUM"))
        spsum = ctx.enter_context(tc.tile_pool(name="spsum", bufs=1, space="PSUM"))

        # ---- persistent tiles -------------------------------------------
        x8g = [
            singles.tile([P, 4, D], FP8, tag=f"x8g{g}", name=f"x8g{g}")
            for g in range(4)
        ]
        qt8 = singles.tile([P, EC, N], FP8, tag="qt8")
        kt8 = singles.tile([P, EC, N], FP8, tag="kt8")
        wq16 = singles.tile([P, EC, D], BF16, tag="wq16")
        wk16 = singles.tile([P, EC, D], BF16, tag="wk16")
        wc16 = singles.tile([P, EC, D], BF16, tag="wc16")
        bqt = singles.tile([P, EC], F32, tag="bqt")
        bkt = singles.tile([P, EC], F32, tag="bkt")
        gamma_b = singles.tile([P, D], F32, tag="gamma_b")
        beta_b = singles.tile([P, D], F32, tag="beta_b")
        ident32 = singles.tile([P, P], F32, tag="ident32")
        maskid16 = singles.tile([P, P], BF16, tag="maskid16")
        ones8 = singles.tile([P, 2, 16], FP8, tag="ones8")
        one32 = singles.tile([1, 1], F32, tag="one32")
        eps_t = singles.tile([P, 1], F32, tag="eps_t")
        cbias_t = singles.tile([P, 1], F32, tag="cbias_t")

        _identity(nc, ident32, 1.0)
        _identity(nc, maskid16, CPRIME)
        nc.gpsimd.memset(ones8, 1.0)
        nc.gpsimd.memset(one32, 1.0)
        nc.gpsimd.memset(eps_t, 1e-5)
        nc.gpsimd.memset(cbias_t, -CBIAS)

        # biases laid out per-partition: b[f] -> [p, fc] with f = fc*128 + p
        nc.sync.dma_start(out=bqt, in_=bq_ext[:].rearrange("(fc p) -> p fc", p=P))
        nc.sync.dma_start(out=bkt, in_=bk_ext[:].rearrange("(fc p) -> p fc", p=P))
        ge = g_ext[:]
        nc.sync.dma_start(
            out=gamma_b,
            in_=bass.AP(tensor=ge.tensor, offset=ge.offset, ap=[[0, P], *ge.ap]),
        )
        bea = be_ext[:]
        nc.sync.dma_start(
            out=beta_b,
            in_=bass.AP(tensor=bea.tensor, offset=bea.offset, ap=[[0, P], *bea.ap]),
        )

        # ---- HAM warmup: dummy matmul burst while the first DMAs land ----
        # PE clock-gate needs ~3.4us of sustained activity to go 1.2->2.4GHz;
        # burn the DMA-wait with throwaway matmuls so prep runs warm.
        warm_ps = sps.tile([P, 512], F32, tag="sps")
        for j in range(56):
            nc.tensor.matmul(
                out=warm_ps[:, (j % 4) * P : (j % 4 + 1) * P],
                lhsT=maskid16,
                rhs=maskid16,
                start=True,
                stop=True,
                skip_group_check=True,
            )

        # ---- stage q/k weights -> bf16 (ACT queue, parallel to X) -------
        # (Wc is loaded after the projections -- not needed until oc0 tail)
        for w_ext, w16 in ((wq_ext, wq16), (wk_ext, wk16)):
            ws = wstage.tile([P, EC, D], F32, tag="wstage")
            nc.scalar.dma_start(
                out=ws, in_=w_ext[:, :].rearrange("(ec p) f -> p ec f", p=P)
            )
            nc.vector.tensor_copy(out=w16, in_=ws)

        # ---- stage x + projections qT[f,i], kT[f,i] ---------------------
        xs_tiles = []
        for g in range(4):
            xs = eoc.tile([P, 4, D], F32, tag="ef")
            nc.sync.dma_start(
                out=xs,
                in_=x_ext[g * 4 * P : (g + 1) * 4 * P, :].rearrange(
                    "(ib p) e -> p ib e", p=P
                ),
            )
            xs_tiles.append(xs)
        for g in range(4):
            xs = xs_tiles[g]
            xt = xtp.tile([P, EC, 512], BF16, tag="xt")
            for ec in range(EC):
                tp = sps.tile([P, 512], F32, tag="sps")
                for k4 in range(4):
                    nc.tensor.transpose(
                        out=tp[:, k4 * P : (k4 + 1) * P],
                        in_=xs[:, k4, ec * P : (ec + 1) * P],
                        identity=ident32,
                    )
                nc.vector.tensor_copy(out=xt[:, ec, :], in_=tp)
            for w16, bt, dst in ((wq16, bqt, qt8), (wk16, bkt, kt8)):
                for fc in range(EC):
                    ps = sps.tile([P, 512], F32, tag="sps")
                    for ec in range(EC):
                        nc.tensor.matmul(
                            out=ps,
                            lhsT=w16[:, ec, fc * P : (fc + 1) * P],
                            rhs=xt[:, ec, :],
                            start=(ec == 0),
                            stop=(ec == EC - 1),
                            skip_group_check=True,
                        )
                    nc.scalar.activation(
                        out=dst[:, fc, g * 512 : (g + 1) * 512],
                        in_=ps,
                        func=mybir.ActivationFunctionType.Identity,
                        bias=bt[:, fc : fc + 1],
                        scale=1.0,
                    )

        # edge loader: one quarter of an o-chunk's mask columns at a time
        def emit_e_quarter(oc, q):
            ef = eoc.tile([P, 4, 512], F32, tag="ef", name=f"ef{oc}{q}")
            nc.sync.dma_start(
                out=ef,
                in_=e_ext[
                    oc * 512 : (oc + 1) * 512, q * 512 : (q + 1) * 512
                ].rearrange("(s p) f -> p s f", p=P),
            )
            e16 = e16p.tile([P, 4, 512], BF16, tag="e16", name=f"e16{oc}{q}")
            # f32->bf16 cast split across DVE and ACT (gpsimd is byte-limited)
            if q % 2 == 0:
                nc.vector.tensor_copy(out=e16, in_=ef)
            else:
                nc.scalar.copy(out=e16, in_=ef)
            return e16

        # oc0 edges pre-issued so the gpsimd DMA ring starts before x8g casts
        e16_pre = [emit_e_quarter(0, q) for q in range(2)]

        for g in range(4):
            nc.gpsimd.tensor_copy(out=x8g[g], in_=xs_tiles[g])

        # deferred Wc staging (first used at the oc0 tail)
        ws = wstage.tile([P, EC, D], F32, tag="wstage")
        nc.scalar.dma_start(
            out=ws, in_=wc_ext[:, :].rearrange("(ec p) f -> p ec f", p=P)
        )
        nc.vector.tensor_copy(out=wc16, in_=ws)

        # ---- main loop over o-chunks ------------------------------------
        for oc in range(OC):
            # residual rows for this chunk (prefetch early)
            xres_tiles = []
            for os4 in range(4):
                r0 = (oc * 4 + os4) * P
                xr = xresp.tile([P, D], F32, tag="xres")
                nc.sync.dma_start(out=xr, in_=x_ext[r0 : r0 + P, :])
                xres_tiles.append(xr)

            if oc == 0:
                e16_q = e16_pre + [emit_e_quarter(0, q) for q in range(2, 4)]
            else:
                e16_q = [emit_e_quarter(oc, q) for q in range(4)]

            ctx_ps = ctxps_pool.tile([P, EC, 512], F32, tag="ctxps")
            s_ps = spsum.tile([1, 512], F32, tag="spsum")

            pt2 = None
            for ib in range(NB):
                e16, il = e16_q[ib // 4], ib % 4
                sp = sps.tile([P, 512], F32, tag="sps")
                for s in range(4):
                    # start=True clears the whole PSUM bank -> only on s==0;
                    # later mask MMs hit has_written=0 and write directly.
                    nc.tensor.matmul(
                        out=sp[:, s * P : (s + 1) * P],
                        lhsT=e16[:, s, il * P : (il + 1) * P],
                        rhs=maskid16,
                        start=(s == 0),
                        stop=False,
                        skip_group_check=True,
                    )
                for dc in (0, 2):
                    nc.tensor.matmul(
                        out=sp,
                        lhsT=kt8[:, dc : dc + 2, ib * P : (ib + 1) * P],
                        rhs=qt8[:, dc : dc + 2, oc * 512 : (oc + 1) * 512],
                        start=False,
                        stop=(dc == 2),
                        perf_mode=mybir.MatmulPerfMode.DoubleRow,
                        skip_group_check=True,
                    )
                if ib % 2 == 0:
                    pt2 = ptp.tile([P, 2, 512], FP8, tag="pt")
                nc.scalar.activation(
                    out=pt2[:, ib % 2, :],
                    in_=sp,
                    func=mybir.ActivationFunctionType.Exp,
                    bias=cbias_t[:, 0:1],
                    scale=ALPHA,
                )
                if ib % 2 == 1:
                    j = (ib % 4) - 1
                    for ec in range(EC):
                        nc.tensor.matmul(
                            out=ctx_ps[:, ec, :],
                            lhsT=x8g[ib // 4][:, j : j + 2, ec * P : (ec + 1) * P],
                            rhs=pt2,
                            start=(ib == 1),
                            stop=(ib == NB - 1),
                            perf_mode=mybir.MatmulPerfMode.DoubleRow,
                            skip_group_check=True,
                        )
                    nc.tensor.matmul(
                        out=s_ps,
                        lhsT=ones8[:, :, 0:1],
                        rhs=pt2,
                        start=(ib == 1),
                        stop=(ib == NB - 1),
                        perf_mode=mybir.MatmulPerfMode.DoubleRow,
                        skip_group_check=True,
                    )

            # unnormalized ctx -> SBUF bf16 (independent of S: overlaps)
            ctx16 = ctxp.tile([P, EC, 512], BF16, tag="ctx16")
            for ec in range(EC):
                if ec % 2 == 0:
                    nc.vector.tensor_copy(out=ctx16[:, ec, :], in_=ctx_ps[:, ec, :])
                else:
                    nc.scalar.copy(out=ctx16[:, ec, :], in_=ctx_ps[:, ec, :])

            s_sb = rowp.tile([1, 512], F32, tag="s_sb")
            nc.vector.tensor_copy(out=s_sb, in_=s_ps)

            # out_pre = ctx_unnorm @ Wc ; scale rows by 1/S ; residual + LN
            # pass 1: x = out_pre/S + xres, batched mean/var
            x_tiles = []
            mv4 = epi.tile([P, 4, 2], F32, tag="mv4")
            s_col = None
            rs_col = None
            for os4 in range(4):
                opre = sps.tile([P, 512], F32, tag="sps")
                for ec in range(EC):
                    nc.tensor.matmul(
                        out=opre,
                        lhsT=ctx16[:, ec, os4 * P : (os4 + 1) * P],
                        rhs=wc16[:, ec, :],
                        start=(ec == 0),
                        stop=(ec == EC - 1),
                        skip_group_check=True,
                    )
                if os4 == 0:
                    # 1/S per-partition: S row -> PE transpose -> recip;
                    # emitted after the first Wc group so PE never waits
                    # on the DVE S-copy.
                    s_col = sps.tile([P, 4], F32, tag="sps")
                    for j in range(4):
                        nc.tensor.matmul(
                            out=s_col[:, j : j + 1],
                            lhsT=s_sb[0:1, j * P : (j + 1) * P],
                            rhs=one32,
                            is_transpose=True,
                            start=(j == 0),
                            stop=(j == 3),
                            skip_group_check=True,
                        )
                    rs_col = rowp.tile([P, 4], F32, tag="rs_col")
                    nc.vector.reciprocal(out=rs_col, in_=s_col)
                t0 = epi.tile([P, D], F32, tag="t0")
                nc.vector.tensor_scalar_mul(t0, opre, rs_col[:, os4 : os4 + 1])
                x_sb = xsbp.tile([P, D], F32, tag="x_sb")
                nc.vector.tensor_add(x_sb, t0, xres_tiles[os4])
                x_tiles.append(x_sb)
                stats = epi.tile([P, 6], F32, tag="stats")
                nc.vector.bn_stats(out=stats, in_=x_sb)
                nc.vector.bn_aggr(out=mv4[:, os4, :], in_=stats)
            # one Sqrt + one reciprocal per o-chunk (avoids ACT table thrash)
            sd4 = epi.tile([P, 4], F32, tag="sd4")
            nc.scalar.activation(
                out=sd4,
                in_=mv4[:, :, 1],
                func=mybir.ActivationFunctionType.Sqrt,
                bias=eps_t[:, 0:1],
                scale=1.0,
            )
            rs4 = epi.tile([P, 4], F32, tag="rs4")
            nc.vector.reciprocal(out=rs4, in_=sd4)
            # pass 2: normalize, gamma/beta, store
            for os4 in range(4):
                t_sb = epi.tile([P, D], F32, tag="t_sb")
                nc.vector.tensor_scalar(
                    t_sb,
                    x_tiles[os4],
                    mv4[:, os4, 0:1],
                    rs4[:, os4 : os4 + 1],
                    mybir.AluOpType.subtract,
                    mybir.AluOpType.mult,
                )
                g_sb = epi.tile([P, D], F32, tag="g_sb")
                nc.gpsimd.tensor_mul(g_sb, t_sb, gamma_b)
                o_sb = epi.tile([P, D], F32, tag="o_sb")
                nc.vector.tensor_add(o_sb, g_sb, beta_b)
                r0 = (oc * 4 + os4) * P
                nc.sync.dma_start(out=out_ext[r0 : r0 + P, :], in_=o_sb)

    _split_excess_waits(nc)
    return nc


_NC_CACHE = None


def kernel(**inputs) -> np.ndarray:
    global _NC_CACHE
    _apply_patches()
    from concourse.bass_utils import run_bass_kernel_spmd

    node_fts = np.ascontiguousarray(np.asarray(inputs["node_fts"], dtype=np.float32))
    rel_edges = np.ascontiguousarray(np.asarray(inputs["rel_edges"], dtype=np.float32))
    shared = {
        k: np.ascontiguousarray(np.asarray(inputs[k], dtype=np.float32))
        for k in ("Wq", "bq", "Wk", "bk", "Wc", "gamma", "beta")
    }
    if _NC_CACHE is None:
        _NC_CACHE = build_nc()
    in_maps = [
        {"node_fts": node_fts[b], "rel_edges": rel_edges[b], **shared}
        for b in range(B)
    ]
    res = run_bass_kernel_spmd(_NC_CACHE, in_maps, core_ids=list(range(B)))
    return np.stack([res.results[b]["out"] for b in range(B)]).astype(np.float32)


# revision 71
# speedup vs baseline: 1.0996x; 1.0315x over previous
"""AttnGCN layer on 8 TRN2 NeuronCores — data-parallel over batch.

Per-core (one sample b):
  q = x @ Wq + bq ; k = x @ Wk + bk            (bf16 PE matmuls)
  sT[i,o] = k_i . q_o  + C'*eT[i,o]            (scores transposed; mask folded
                                                into PSUM via lhsT=e-block
                                                matmuls against a scaled
                                                identity — transposes e free)
  pT = exp(alpha*sT - C)                        (ACT, masked entries -> ~0)
  S[o] = sum_i pT[i,o]                          (ones-vector PE matmul)
  ctxT[e,o] = sum_i x16[i,e] * pT[i,o]          (PE, accumulated over i-blocks)
  ctxN = ctxT * (1/S broadcast)                 (PE broadcast + DVE mul)
  out_pre[o,c] = sum_e ctxN[e,o] * Wc[e,c]      (PE)
  x = x + out_pre ; LayerNorm(x)*gamma + beta   (DVE bn_stats/bn_aggr epilogue)

Self-contained: hardcodes shapes from the problem spec.
"""

import math
from contextlib import ExitStack

import numpy as np

import concourse.bass as bass
import concourse.tile as tile
from concourse import mybir
from concourse.vector_clock import ScopedClock

F32 = mybir.dt.float32
F32R = mybir.dt.float32r
BF16 = mybir.dt.bfloat16
FP8 = mybir.dt.float8e4

B = 8
N = 2048
D = 512
P = 128
NB = N // P       # 16 i-blocks
EC = D // P       # 4 chunks of the embed/dff dim
OC = N // 512     # 4 o-chunks of 512 attn rows
ALPHA = 1.0 / math.sqrt(D)
CPRIME = 1024.0           # mask scale inside PSUM (exactly representable)
SHIFT = 3.0               # softmax-invariant shift keeping exp() in fp8 range
CBIAS = CPRIME * ALPHA + SHIFT  # subtracted in the exp bias


# ---------------------------------------------------------------------------
# Workaround: walrus CoreV3 rejects >2 sem waits on the TileContext final
# drain ("Too many sync wait commands"). Hoist waits onto preceding nops.
def _patched_drain_and_barrier(self, tick_clock, wait_clock):
    nc = self.nc
    carrier = nc.sync.nop(nofuse=True)
    wait_clock.add_sem_waits(carrier.ins, ScopedClock({None: tick_clock.global_clock}))
    si = carrier.ins.sync_info
    waits = list(si.on_wait) if si and si.on_wait else []
    if len(waits) > 1:
        si.on_wait = waits[:1]
        for w in waits[1:]:
            n2 = nc.sync.nop(nofuse=True)
            n2.ins.sync_info = mybir.SyncInfo(on_wait=[w], on_update=[])
    nc.sync.drain()
    nc.all_engine_barrier()
    assert self.sems is not None
    popped = nc._tile_sem_poison_stack.pop()
    assert popped is self._sem_poison
    nc.clear_and_free_semaphores(list(self.sems.allocated().values()))
    nc.all_engine_barrier()


def _apply_patches():
    tile.TileContext._drain_and_barrier = _patched_drain_and_barrier


def _split_excess_waits(nc, limit=1):
    """walrus CoreV2/V3 codegen rejects instructions with >2 sem waits;
    hoist excess waits onto same-engine no-ops inserted just before."""
    n = 0
    for fn in nc.m.functions:
        for blk in fn.blocks:
            out = []
            changed = False
            for inst in blk.instructions:
                si = inst.sync_info
                waits = list(si.on_wait) if si and si.on_wait else []
                if len(waits) > limit:
                    keep = waits[-limit:]
                    for w in waits[:-limit]:
                        n += 1
                        nop = mybir.InstNoOp(name=f"I-wsplit-{n}", ins=[], outs=[])
                        nop.engine = inst.engine
                        nop.sync_info = mybir.SyncInfo(on_wait=[w], on_update=[])
                        out.append(nop)
                    si.on_wait = keep
                    changed = True
                out.append(inst)
            if changed:
                blk.instructions = out
    return n


def _r(ap):
    """View an f32 AP as float32r for full-rate PE matmuls."""
    return ap.bitcast(F32R)


def _identity(nc, ap, diag):
    nc.gpsimd.memset(ap, 0.0)
    nc.gpsimd.affine_select(
        out=ap,
        in_=ap,
        compare_op=mybir.AluOpType.not_equal,
        fill=diag,
        base=0,
        pattern=[[-1, ap.shape[0]]],
        channel_multiplier=1,
    )


def build_nc():
    nc = bass.Bass()
    x_ext = nc.declare_dram_parameter("node_fts", [N, D], F32, isOutput=False)
    e_ext = nc.declare_dram_parameter("rel_edges", [N, N], F32, isOutput=False)
    wq_ext = nc.declare_dram_parameter("Wq", [D, D], F32, isOutput=False)
    bq_ext = nc.declare_dram_parameter("bq", [D], F32, isOutput=False)
    wk_ext = nc.declare_dram_parameter("Wk", [D, D], F32, isOutput=False)
    bk_ext = nc.declare_dram_parameter("bk", [D], F32, isOutput=False)
    wc_ext = nc.declare_dram_parameter("Wc", [D, D], F32, isOutput=False)
    g_ext = nc.declare_dram_parameter("gamma", [D], F32, isOutput=False)
    be_ext = nc.declare_dram_parameter("beta", [D], F32, isOutput=False)
    out_ext = nc.declare_dram_parameter("out", [N, D], F32, isOutput=True)

    with tile.TileContext(nc) as tc, ExitStack() as ctx:
        singles = ctx.enter_context(tc.tile_pool(name="singles", bufs=1))
        wstage = ctx.enter_context(tc.tile_pool(name="wstage", bufs=2))
        xtp = ctx.enter_context(tc.tile_pool(name="xtp", bufs=2))
        eoc = ctx.enter_context(tc.tile_pool(name="eoc", bufs=6))
        e16p = ctx.enter_context(tc.tile_pool(name="e16p", bufs=4))
        ptp = ctx.enter_context(tc.tile_pool(name="ptp", bufs=4))
        ctxp = ctx.enter_context(tc.tile_pool(name="ctxp", bufs=2))
        rowp = ctx.enter_context(tc.tile_pool(name="rowp", bufs=2))
        epi = ctx.enter_context(tc.tile_pool(name="epi", bufs=2))
        xsbp = ctx.enter_context(tc.tile_pool(name="xsbp", bufs=5))
        xresp = ctx.enter_context(tc.tile_pool(name="xresp", bufs=6))
        sps = ctx.enter_context(tc.tile_pool(name="sps", bufs=3, space="PSUM"))
        ctxps_pool = ctx.enter_context(tc.tile_pool(name="ctxps", bufs=1, space="PSUM"))
        spsum = ctx.enter_context(tc.tile_pool(name="spsum", bufs=1, space="PSUM"))

        # ---- persistent tiles -------------------------------------------
        x8g = [
            singles.tile([P, 4, D], FP8, tag=f"x8g{g}", name=f"x8g{g}")
            for g in range(4)
        ]
        qt8 = singles.tile([P, EC, N], FP8, tag="qt8")
        kt8 = singles.tile([P, EC, N], FP8, tag="kt8")
        wq16 = singles.tile([P, EC, D], BF16, tag="wq16")
        wk16 = singles.tile([P, EC, D], BF16, tag="wk16")
        wc16 = singles.tile([P, EC, D], BF16, tag="wc16")
        bqt = singles.tile([P, EC], F32, tag="bqt")
        bkt = singles.tile([P, EC], F32, tag="bkt")
        gamma_b = singles.tile([P, D], F32, tag="gamma_b")
        beta_b = singles.tile([P, D], F32, tag="beta_b")
        ident32 = singles.tile([P, P], F32, tag="ident32")
        maskid16 = singles.tile([P, P], BF16, tag="maskid16")
        ones8 = singles.tile([P, 2, 16], FP8, tag="ones8")
        one32 = singles.tile([1, 1], F32, tag="one32")
        eps_t = singles.tile([P, 1], F32, tag="eps_t")
        cbias_t = singles.tile([P, 1], F32, tag="cbias_t")

        _identity(nc, ident32, 1.0)
        _identity(nc, maskid16, CPRIME)
        nc.gpsimd.memset(ones8, 1.0)
        nc.gpsimd.memset(one32, 1.0)
        nc.gpsimd.memset(eps_t, 1e-5)
        nc.gpsimd.memset(cbias_t, -CBIAS)

        # biases laid out per-partition: b[f] -> [p, fc] with f = fc*128 + p
        nc.sync.dma_start(out=bqt, in_=bq_ext[:].rearrange("(fc p) -> p fc", p=P))
        nc.sync.dma_start(out=bkt, in_=bk_ext[:].rearrange("(fc p) -> p fc", p=P))
        ge = g_ext[:]
        nc.sync.dma_start(
            out=gamma_b,
            in_=bass.AP(tensor=ge.tensor, offset=ge.offset, ap=[[0, P], *ge.ap]),
        )
        bea = be_ext[:]
        nc.sync.dma_start(
            out=beta_b,
            in_=bass.AP(tensor=bea.tensor, offset=bea.offset, ap=[[0, P], *bea.ap]),
        )

        # ---- HAM warmup: dummy matmul burst while the first DMAs land ----
        # PE clock-gate needs ~3.4us of sustained activity to go 1.2->2.4GHz;
        # burn the DMA-wait with throwaway matmuls so prep runs warm.
        warm_ps = sps.tile([P, 512], F32, tag="sps")
        for j in range(104):
            nc.tensor.matmul(
                out=warm_ps[:, (j % 4) * P : (j % 4 + 1) * P],
                lhsT=maskid16,
                rhs=maskid16,
                start=True,
                stop=True,
                skip_group_check=True,
            )

        # ---- stage q/k weights -> bf16 (ACT queue, parallel to X) -------
        # (Wc is loaded after the projections -- not needed until oc0 tail)
        for w_ext, w16 in ((wq_ext, wq16), (wk_ext, wk16)):
            ws = wstage.tile([P, EC, D], F32, tag="wstage")
            nc.scalar.dma_start(
                out=ws, in_=w_ext[:, :].rearrange("(ec p) f -> p ec f", p=P)
            )
            nc.vector.tensor_copy(out=w16, in_=ws)

        # ---- stage x + projections qT[f,i], kT[f,i] ---------------------
        xs_tiles = []
        for g in range(4):
            xs = eoc.tile([P, 4, D], F32, tag="ef")
            nc.sync.dma_start(
                out=xs,
                in_=x_ext[g * 4 * P : (g + 1) * 4 * P, :].rearrange(
                    "(ib p) e -> p ib e", p=P
                ),
            )
            xs_tiles.append(xs)
        for g in range(4):
            xs = xs_tiles[g]
            xt = xtp.tile([P, EC, 512], BF16, tag="xt")
            for ec in range(EC):
                tp = sps.tile([P, 512], F32, tag="sps")
                for k4 in range(4):
                    nc.tensor.transpose(
                        out=tp[:, k4 * P : (k4 + 1) * P],
                        in_=xs[:, k4, ec * P : (ec + 1) * P],
                        identity=ident32,
                    )
                nc.vector.tensor_copy(out=xt[:, ec, :], in_=tp)
            for w16, bt, dst in ((wq16, bqt, qt8), (wk16, bkt, kt8)):
                for fc in range(EC):
                    ps = sps.tile([P, 512], F32, tag="sps")
                    for ec in range(EC):
                        nc.tensor.matmul(
                            out=ps,
                            lhsT=w16[:, ec, fc * P : (fc + 1) * P],
                            rhs=xt[:, ec, :],
                            start=(ec == 0),
                            stop=(ec == EC - 1),
                            skip_group_check=True,
                        )
                    nc.scalar.activation(
                        out=dst[:, fc, g * 512 : (g + 1) * 512],
                        in_=ps,
                        func=mybir.ActivationFunctionType.Identity,
                        bias=bt[:, fc : fc + 1],
                        scale=1.0,
                    )

        # edge loader: one quarter of an o-chunk's mask columns at a time
        def emit_e_quarter(oc, q):
            ef = eoc.tile([P, 4, 512], F32, tag="ef", name=f"ef{oc}{q}")
            nc.sync.dma_start(
                out=ef,
                in_=e_ext[
                    oc * 512 : (oc + 1) * 512, q * 512 : (q + 1) * 512
                ].rearrange("(s p) f -> p s f", p=P),
            )
            e16 = e16p.tile([P, 4, 512], BF16, tag="e16", name=f"e16{oc}{q}")
            # f32->bf16 cast split across DVE and ACT (gpsimd is byte-limited)
            if q % 2 == 0:
                nc.vector.tensor_copy(out=e16, in_=ef)
            else:
                nc.scalar.copy(out=e16, in_=ef)
            return e16

        # oc0 edges pre-issued so the gpsimd DMA ring starts before x8g casts
        e16_pre = [emit_e_quarter(0, q) for q in range(2)]

        for g in range(4):
            nc.gpsimd.tensor_copy(out=x8g[g], in_=xs_tiles[g])

        # deferred Wc staging (first used at the oc0 tail)
        ws = wstage.tile([P, EC, D], F32, tag="wstage")
        nc.scalar.dma_start(
            out=ws, in_=wc_ext[:, :].rearrange("(ec p) f -> p ec f", p=P)
        )
        nc.vector.tensor_copy(out=wc16, in_=ws)

        # ---- main loop over o-chunks ------------------------------------
        for oc in range(OC):
            # residual rows for this chunk (prefetch early)
            xres_tiles = []
            for os4 in range(4):
                r0 = (oc * 4 + os4) * P
                xr = xresp.tile([P, D], F32, tag="xres")
                nc.sync.dma_start(out=xr, in_=x_ext[r0 : r0 + P, :])
                xres_tiles.append(xr)

            if oc == 0:
                e16_q = e16_pre + [emit_e_quarter(0, q) for q in range(2, 4)]
            else:
                e16_q = [emit_e_quarter(oc, q) for q in range(4)]

            ctx_ps = ctxps_pool.tile([P, EC, 512], F32, tag="ctxps")
            s_ps = spsum.tile([1, 512], F32, tag="spsum")

            pt2 = None
            for ib in range(NB):
                e16, il = e16_q[ib // 4], ib % 4
                sp = sps.tile([P, 512], F32, tag="sps")
                for s in range(4):
                    # start=True clears the whole PSUM bank -> only on s==0;
                    # later mask MMs hit has_written=0 and write directly.
                    nc.tensor.matmul(
                        out=sp[:, s * P : (s + 1) * P],
                        lhsT=e16[:, s, il * P : (il + 1) * P],
                        rhs=maskid16,
                        start=(s == 0),
                        stop=False,
                        skip_group_check=True,
                    )
                for dc in (0, 2):
                    nc.tensor.matmul(
                        out=sp,
                        lhsT=kt8[:, dc : dc + 2, ib * P : (ib + 1) * P],
                        rhs=qt8[:, dc : dc + 2, oc * 512 : (oc + 1) * 512],
                        start=False,
                        stop=(dc == 2),
                        perf_mode=mybir.MatmulPerfMode.DoubleRow,
                        skip_group_check=True,
                    )
                if ib % 2 == 0:
                    pt2 = ptp.tile([P, 2, 512], FP8, tag="pt")
                nc.scalar.activation(
                    out=pt2[:, ib % 2, :],
                    in_=sp,
                    func=mybir.ActivationFunctionType.Exp,
                    bias=cbias_t[:, 0:1],
                    scale=ALPHA,
                )
                if ib % 2 == 1:
                    j = (ib % 4) - 1
                    for ec in range(EC):
                        nc.tensor.matmul(
                            out=ctx_ps[:, ec, :],
                            lhsT=x8g[ib // 4][:, j : j + 2, ec * P : (ec + 1) * P],
                            rhs=pt2,
                            start=(ib == 1),
                            stop=(ib == NB - 1),
                            perf_mode=mybir.MatmulPerfMode.DoubleRow,
                            skip_group_check=True,
                        )
                    nc.tensor.matmul(
                        out=s_ps,
                        lhsT=ones8[:, :, 0:1],
                        rhs=pt2,
                        start=(ib == 1),
                        stop=(ib == NB - 1),
                        perf_mode=mybir.MatmulPerfMode.DoubleRow,
                        skip_group_check=True,
                    )

            # unnormalized ctx -> SBUF bf16 (independent of S: overlaps)
            ctx16 = ctxp.tile([P, EC, 512], BF16, tag="ctx16")
            for ec in range(EC):
                if ec % 2 == 0:
                    nc.vector.tensor_copy(out=ctx16[:, ec, :], in_=ctx_ps[:, ec, :])
                else:
                    nc.scalar.copy(out=ctx16[:, ec, :], in_=ctx_ps[:, ec, :])

            s_sb = rowp.tile([1, 512], F32, tag="s_sb")
            nc.vector.tensor_copy(out=s_sb, in_=s_ps)

            # out_pre = ctx_unnorm @ Wc ; scale rows by 1/S ; residual + LN
            # pass 1: x = out_pre/S + xres, batched mean/var
            x_tiles = []
            mv4 = epi.tile([P, 4, 2], F32, tag="mv4")
            s_col = None
            rs_col = None
            for os4 in range(4):
                opre = sps.tile([P, 512], F32, tag="sps")
                for ec in range(EC):
                    nc.tensor.matmul(
                        out=opre,
                        lhsT=ctx16[:, ec, os4 * P : (os4 + 1) * P],
                        rhs=wc16[:, ec, :],
                        start=(ec == 0),
                        stop=(ec == EC - 1),
                        skip_group_check=True,
                    )
                if os4 == 0:
                    # 1/S per-partition: S row -> PE transpose -> recip;
                    # emitted after the first Wc group so PE never waits
                    # on the DVE S-copy.
                    s_col = sps.tile([P, 4], F32, tag="sps")
                    for j in range(4):
                        nc.tensor.matmul(
                            out=s_col[:, j : j + 1],
                            lhsT=s_sb[0:1, j * P : (j + 1) * P],
                            rhs=one32,
                            is_transpose=True,
                            start=(j == 0),
                            stop=(j == 3),
                            skip_group_check=True,
                        )
                    rs_col = rowp.tile([P, 4], F32, tag="rs_col")
                    nc.vector.reciprocal(out=rs_col, in_=s_col)
                t0 = epi.tile([P, D], F32, tag="t0")
                nc.scalar.mul(t0, opre, rs_col[:, os4 : os4 + 1])
                x_sb = xsbp.tile([P, D], F32, tag="x_sb")
                nc.vector.tensor_add(x_sb, t0, xres_tiles[os4])
                x_tiles.append(x_sb)
                stats = epi.tile([P, 6], F32, tag="stats")
                nc.vector.bn_stats(out=stats, in_=x_sb)
                nc.vector.bn_aggr(out=mv4[:, os4, :], in_=stats)
            # one Sqrt + one reciprocal per o-chunk (avoids ACT table thrash)
            sd4 = epi.tile([P, 4], F32, tag="sd4")
            nc.scalar.activation(
                out=sd4,
                in_=mv4[:, :, 1],
                func=mybir.ActivationFunctionType.Sqrt,
                bias=eps_t[:, 0:1],
                scale=1.0,
            )
            rs4 = epi.tile([P, 4], F32, tag="rs4")
            nc.vector.reciprocal(out=rs4, in_=sd4)
            nb4 = epi.tile([P, 4], F32, tag="nb4")
            nc.vector.tensor_tensor(nb4, mv4[:, :, 0], rs4, mybir.AluOpType.mult)
            nc.vector.tensor_scalar_mul(nb4, nb4, -1.0)
            # pass 2: normalize on ACT ((x - mu)*rs as x*rs + (-mu*rs)),
            # gamma on gpsimd, beta on DVE -- three-engine pipeline
            for os4 in range(4):
                t_sb = epi.tile([P, D], F32, tag="t_sb")
                nc.scalar.activation(
                    out=t_sb,
                    in_=x_tiles[os4],
                    func=mybir.ActivationFunctionType.Identity,
                    bias=nb4[:, os4 : os4 + 1],
                    scale=rs4[:, os4 : os4 + 1],
                )
                g_sb = epi.tile([P, D], F32, tag="g_sb")
                nc.gpsimd.tensor_mul(g_sb, t_sb, gamma_b)
                o_sb = epi.tile([P, D], F32, tag="o_sb")
                nc.vector.tensor_add(o_sb, g_sb, beta_b)
                r0 = (oc * 4 + os4) * P
                nc.sync.dma_start(out=out_ext[r0 : r0 + P, :], in_=o_sb)

    _split_excess_waits(nc)
    return nc


_NC_CACHE = None


def kernel(**inputs) -> np.ndarray:
    global _NC_CACHE
    _apply_patches()
    from concourse.bass_utils import run_bass_kernel_spmd

    node_fts = np.ascontiguousarray(np.asarray(inputs["node_fts"], dtype=np.float32))
    rel_edges = np.ascontiguousarray(np.asarray(inputs["rel_edges"], dtype=np.float32))
    shared = {
        k: np.ascontiguousarray(np.asarray(inputs[k], dtype=np.float32))
        for k in ("Wq", "bq", "Wk", "bk", "Wc", "gamma", "beta")
    }
    if _NC_CACHE is None:
        _NC_CACHE = build_nc()
    in_maps = [
        {"node_fts": node_fts[b], "rel_edges": rel_edges[b], **shared}
        for b in range(B)
    ]
    res = run_bass_kernel_spmd(_NC_CACHE, in_maps, core_ids=list(range(B)))
    return np.stack([res.results[b]["out"] for b in range(B)]).astype(np.float32)


# revision 74
# speedup vs baseline: 1.1361x; 1.0332x over previous
"""AttnGCN layer on 8 TRN2 NeuronCores — data-parallel over batch.

Per-core (one sample b):
  q = x @ Wq + bq ; k = x @ Wk + bk            (bf16 PE matmuls)
  sT[i,o] = k_i . q_o  + C'*eT[i,o]            (scores transposed; mask folded
                                                into PSUM via lhsT=e-block
                                                matmuls against a scaled
                                                identity — transposes e free)
  pT = exp(alpha*sT - C)                        (ACT, masked entries -> ~0)
  S[o] = sum_i pT[i,o]                          (ones-vector PE matmul)
  ctxT[e,o] = sum_i x16[i,e] * pT[i,o]          (PE, accumulated over i-blocks)
  ctxN = ctxT * (1/S broadcast)                 (PE broadcast + DVE mul)
  out_pre[o,c] = sum_e ctxN[e,o] * Wc[e,c]      (PE)
  x = x + out_pre ; LayerNorm(x)*gamma + beta   (DVE bn_stats/bn_aggr epilogue)

Self-contained: hardcodes shapes from the problem spec.
"""

import math
from contextlib import ExitStack

import numpy as np

import concourse.bass as bass
import concourse.tile as tile
from concourse import mybir
from concourse.vector_clock import ScopedClock

F32 = mybir.dt.float32
F32R = mybir.dt.float32r
BF16 = mybir.dt.bfloat16
FP8 = mybir.dt.float8e4

B = 8
N = 2048
D = 512
P = 128
NB = N // P       # 16 i-blocks
EC = D // P       # 4 chunks of the embed/dff dim
OC = N // 512     # 4 o-chunks of 512 attn rows
ALPHA = 1.0 / math.sqrt(D)
CPRIME = 1024.0           # mask scale inside PSUM (exactly representable)
SHIFT = 3.0               # softmax-invariant shift keeping exp() in fp8 range
CBIAS = CPRIME * ALPHA + SHIFT  # subtracted in the exp bias


# ---------------------------------------------------------------------------
# Workaround: walrus CoreV3 rejects >2 sem waits on the TileContext final
# drain ("Too many sync wait commands"). Hoist waits onto preceding nops.
def _patched_drain_and_barrier(self, tick_clock, wait_clock):
    nc = self.nc
    carrier = nc.sync.nop(nofuse=True)
    wait_clock.add_sem_waits(carrier.ins, ScopedClock({None: tick_clock.global_clock}))
    si = carrier.ins.sync_info
    waits = list(si.on_wait) if si and si.on_wait else []
    if len(waits) > 1:
        si.on_wait = waits[:1]
        for w in waits[1:]:
            n2 = nc.sync.nop(nofuse=True)
            n2.ins.sync_info = mybir.SyncInfo(on_wait=[w], on_update=[])
    nc.sync.drain()
    nc.all_engine_barrier()
    assert self.sems is not None
    popped = nc._tile_sem_poison_stack.pop()
    assert popped is self._sem_poison
    nc.clear_and_free_semaphores(list(self.sems.allocated().values()))
    nc.all_engine_barrier()


def _apply_patches():
    tile.TileContext._drain_and_barrier = _patched_drain_and_barrier


def _split_excess_waits(nc, limit=1):
    """walrus CoreV2/V3 codegen rejects instructions with >2 sem waits;
    hoist excess waits onto same-engine no-ops inserted just before."""
    n = 0
    for fn in nc.m.functions:
        for blk in fn.blocks:
            out = []
            changed = False
            for inst in blk.instructions:
                si = inst.sync_info
                waits = list(si.on_wait) if si and si.on_wait else []
                if len(waits) > limit:
                    keep = waits[-limit:]
                    for w in waits[:-limit]:
                        n += 1
                        nop = mybir.InstNoOp(name=f"I-wsplit-{n}", ins=[], outs=[])
                        nop.engine = inst.engine
                        nop.sync_info = mybir.SyncInfo(on_wait=[w], on_update=[])
                        out.append(nop)
                    si.on_wait = keep
                    changed = True
                out.append(inst)
            if changed:
                blk.instructions = out
    return n


def _r(ap):
    """View an f32 AP as float32r for full-rate PE matmuls."""
    return ap.bitcast(F32R)


def _identity(nc, ap, diag):
    nc.gpsimd.memset(ap, 0.0)
    nc.gpsimd.affine_select(
        out=ap,
        in_=ap,
        compare_op=mybir.AluOpType.not_equal,
        fill=diag,
        base=0,
        pattern=[[-1, ap.shape[0]]],
        channel_multiplier=1,
    )


def build_nc():
    nc = bass.Bass()
    x_ext = nc.declare_dram_parameter("node_fts", [N, D], F32, isOutput=False)
    e_ext = nc.declare_dram_parameter("rel_edges", [N, N], F32, isOutput=False)
    wq_ext = nc.declare_dram_parameter("Wq", [D, D], F32, isOutput=False)
    bq_ext = nc.declare_dram_parameter("bq", [D], F32, isOutput=False)
    wk_ext = nc.declare_dram_parameter("Wk", [D, D], F32, isOutput=False)
    bk_ext = nc.declare_dram_parameter("bk", [D], F32, isOutput=False)
    wc_ext = nc.declare_dram_parameter("Wc", [D, D], F32, isOutput=False)
    g_ext = nc.declare_dram_parameter("gamma", [D], F32, isOutput=False)
    be_ext = nc.declare_dram_parameter("beta", [D], F32, isOutput=False)
    out_ext = nc.declare_dram_parameter("out", [N, D], F32, isOutput=True)

    with tile.TileContext(nc) as tc, ExitStack() as ctx:
        singles = ctx.enter_context(tc.tile_pool(name="singles", bufs=1))
        wstage = ctx.enter_context(tc.tile_pool(name="wstage", bufs=2))
        xtp = ctx.enter_context(tc.tile_pool(name="xtp", bufs=2))
        eoc = ctx.enter_context(tc.tile_pool(name="eoc", bufs=6))
        e16p = ctx.enter_context(tc.tile_pool(name="e16p", bufs=4))
        ptp = ctx.enter_context(tc.tile_pool(name="ptp", bufs=4))
        ctxp = ctx.enter_context(tc.tile_pool(name="ctxp", bufs=2))
        rowp = ctx.enter_context(tc.tile_pool(name="rowp", bufs=2))
        epi = ctx.enter_context(tc.tile_pool(name="epi", bufs=2))
        xsbp = ctx.enter_context(tc.tile_pool(name="xsbp", bufs=5))
        xresp = ctx.enter_context(tc.tile_pool(name="xresp", bufs=6))
        sps = ctx.enter_context(tc.tile_pool(name="sps", bufs=3, space="PSUM"))
        ctxps_pool = ctx.enter_context(tc.tile_pool(name="ctxps", bufs=1, space="PSUM"))
        spsum = ctx.enter_context(tc.tile_pool(name="spsum", bufs=1, space="PSUM"))

        # ---- persistent tiles -------------------------------------------
        x8g = [
            singles.tile([P, 4, D], FP8, tag=f"x8g{g}", name=f"x8g{g}")
            for g in range(4)
        ]
        qt8 = singles.tile([P, EC, N], FP8, tag="qt8")
        kt8 = singles.tile([P, EC, N], FP8, tag="kt8")
        wq16 = singles.tile([P, EC, D], BF16, tag="wq16")
        wk16 = singles.tile([P, EC, D], BF16, tag="wk16")
        wc16 = singles.tile([P, EC, D], BF16, tag="wc16")
        bqt = singles.tile([P, EC], F32, tag="bqt")
        bkt = singles.tile([P, EC], F32, tag="bkt")
        gamma_b = singles.tile([P, D], F32, tag="gamma_b")
        beta_b = singles.tile([P, D], F32, tag="beta_b")
        ident32 = singles.tile([P, P], F32, tag="ident32")
        maskid16 = singles.tile([P, P], BF16, tag="maskid16")
        ones8 = singles.tile([P, 2, 16], FP8, tag="ones8")
        one32 = singles.tile([1, 1], F32, tag="one32")
        eps_t = singles.tile([P, 1], F32, tag="eps_t")
        cbias_t = singles.tile([P, 1], F32, tag="cbias_t")

        _identity(nc, ident32, 1.0)
        _identity(nc, maskid16, CPRIME)
        nc.gpsimd.memset(ones8, 1.0)
        nc.gpsimd.memset(one32, 1.0)
        nc.gpsimd.memset(eps_t, 1e-5)
        nc.gpsimd.memset(cbias_t, -CBIAS)

        # biases laid out per-partition: b[f] -> [p, fc] with f = fc*128 + p
        nc.sync.dma_start(out=bqt, in_=bq_ext[:].rearrange("(fc p) -> p fc", p=P))
        nc.sync.dma_start(out=bkt, in_=bk_ext[:].rearrange("(fc p) -> p fc", p=P))
        ge = g_ext[:]
        nc.sync.dma_start(
            out=gamma_b,
            in_=bass.AP(tensor=ge.tensor, offset=ge.offset, ap=[[0, P], *ge.ap]),
        )
        bea = be_ext[:]
        nc.sync.dma_start(
            out=beta_b,
            in_=bass.AP(tensor=bea.tensor, offset=bea.offset, ap=[[0, P], *bea.ap]),
        )

        # ---- HAM warmup: dummy matmul burst while the first DMAs land ----
        # PE clock-gate needs ~3.4us of sustained activity to go 1.2->2.4GHz;
        # burn the DMA-wait with throwaway matmuls so prep runs warm.
        warm_ps = sps.tile([P, 512], F32, tag="sps")
        for j in range(104):
            nc.tensor.matmul(
                out=warm_ps[:, (j % 4) * P : (j % 4 + 1) * P],
                lhsT=maskid16,
                rhs=maskid16,
                start=True,
                stop=True,
                skip_group_check=True,
            )

        # ---- stage q/k weights -> bf16 (ACT queue, parallel to X) -------
        # (Wc is loaded after the projections -- not needed until oc0 tail)
        for w_ext, w16 in ((wq_ext, wq16), (wk_ext, wk16)):
            ws = wstage.tile([P, EC, D], F32, tag="wstage")
            nc.scalar.dma_start(
                out=ws, in_=w_ext[:, :].rearrange("(ec p) f -> p ec f", p=P)
            )
            nc.vector.tensor_copy(out=w16, in_=ws)

        # ---- stage x + projections qT[f,i], kT[f,i] ---------------------
        xs_tiles = []
        for g in range(4):
            xs = eoc.tile([P, 4, D], F32, tag="ef")
            nc.sync.dma_start(
                out=xs,
                in_=x_ext[g * 4 * P : (g + 1) * 4 * P, :].rearrange(
                    "(ib p) e -> p ib e", p=P
                ),
            )
            xs_tiles.append(xs)
        for g in range(4):
            xs = xs_tiles[g]
            xt = xtp.tile([P, EC, 512], BF16, tag="xt")
            for ec in range(EC):
                tp = sps.tile([P, 512], F32, tag="sps")
                for k4 in range(4):
                    nc.tensor.transpose(
                        out=tp[:, k4 * P : (k4 + 1) * P],
                        in_=xs[:, k4, ec * P : (ec + 1) * P],
                        identity=ident32,
                    )
                nc.vector.tensor_copy(out=xt[:, ec, :], in_=tp)
            for w16, bt, dst in ((wq16, bqt, qt8), (wk16, bkt, kt8)):
                for fc in range(EC):
                    ps = sps.tile([P, 512], F32, tag="sps")
                    for ec in range(EC):
                        nc.tensor.matmul(
                            out=ps,
                            lhsT=w16[:, ec, fc * P : (fc + 1) * P],
                            rhs=xt[:, ec, :],
                            start=(ec == 0),
                            stop=(ec == EC - 1),
                            skip_group_check=True,
                        )
                    nc.scalar.activation(
                        out=dst[:, fc, g * 512 : (g + 1) * 512],
                        in_=ps,
                        func=mybir.ActivationFunctionType.Identity,
                        bias=bt[:, fc : fc + 1],
                        scale=1.0,
                    )

        # edge loader: one quarter of an o-chunk's mask columns at a time
        def emit_e_quarter(oc, q):
            ef = eoc.tile([P, 4, 512], F32, tag="ef", name=f"ef{oc}{q}")
            nc.sync.dma_start(
                out=ef,
                in_=e_ext[
                    oc * 512 : (oc + 1) * 512, q * 512 : (q + 1) * 512
                ].rearrange("(s p) f -> p s f", p=P),
            )
            e16 = e16p.tile([P, 4, 512], BF16, tag="e16", name=f"e16{oc}{q}")
            # f32->bf16 cast on DVE only: a Copy on ACT between Exps would
            # force a ~1.3us activation-table reload on the exp critical path
            nc.vector.tensor_copy(out=e16, in_=ef)
            return e16

        # oc0 edges pre-issued so the gpsimd DMA ring starts before x8g casts
        e16_pre = [emit_e_quarter(0, q) for q in range(2)]

        for g in range(4):
            nc.gpsimd.tensor_copy(out=x8g[g], in_=xs_tiles[g])

        # deferred Wc staging (first used at the oc0 tail)
        ws = wstage.tile([P, EC, D], F32, tag="wstage")
        nc.scalar.dma_start(
            out=ws, in_=wc_ext[:, :].rearrange("(ec p) f -> p ec f", p=P)
        )
        nc.vector.tensor_copy(out=wc16, in_=ws)

        # ---- main loop over o-chunks ------------------------------------
        for oc in range(OC):
            # residual rows for this chunk (prefetch early)
            xres_tiles = []
            for os4 in range(4):
                r0 = (oc * 4 + os4) * P
                xr = xresp.tile([P, D], F32, tag="xres")
                nc.sync.dma_start(out=xr, in_=x_ext[r0 : r0 + P, :])
                xres_tiles.append(xr)

            if oc == 0:
                e16_q = e16_pre + [emit_e_quarter(0, q) for q in range(2, 4)]
            else:
                e16_q = [emit_e_quarter(oc, q) for q in range(4)]

            ctx_ps = ctxps_pool.tile([P, EC, 512], F32, tag="ctxps")
            s_ps = spsum.tile([1, 512], F32, tag="spsum")

            pt2 = None
            for ib in range(NB):
                e16, il = e16_q[ib // 4], ib % 4
                sp = sps.tile([P, 512], F32, tag="sps")
                for s in range(4):
                    # start=True clears the whole PSUM bank -> only on s==0;
                    # later mask MMs hit has_written=0 and write directly.
                    nc.tensor.matmul(
                        out=sp[:, s * P : (s + 1) * P],
                        lhsT=e16[:, s, il * P : (il + 1) * P],
                        rhs=maskid16,
                        start=(s == 0),
                        stop=False,
                        skip_group_check=True,
                    )
                for dc in (0, 2):
                    nc.tensor.matmul(
                        out=sp,
                        lhsT=kt8[:, dc : dc + 2, ib * P : (ib + 1) * P],
                        rhs=qt8[:, dc : dc + 2, oc * 512 : (oc + 1) * 512],
                        start=False,
                        stop=(dc == 2),
                        perf_mode=mybir.MatmulPerfMode.DoubleRow,
                        skip_group_check=True,
                    )
                if ib % 2 == 0:
                    pt2 = ptp.tile([P, 2, 512], FP8, tag="pt")
                nc.scalar.activation(
                    out=pt2[:, ib % 2, :],
                    in_=sp,
                    func=mybir.ActivationFunctionType.Exp,
                    bias=cbias_t[:, 0:1],
                    scale=ALPHA,
                )
                if ib % 2 == 1:
                    j = (ib % 4) - 1
                    for ec in range(EC):
                        nc.tensor.matmul(
                            out=ctx_ps[:, ec, :],
                            lhsT=x8g[ib // 4][:, j : j + 2, ec * P : (ec + 1) * P],
                            rhs=pt2,
                            start=(ib == 1),
                            stop=(ib == NB - 1),
                            perf_mode=mybir.MatmulPerfMode.DoubleRow,
                            skip_group_check=True,
                        )
                    nc.tensor.matmul(
                        out=s_ps,
                        lhsT=ones8[:, :, 0:1],
                        rhs=pt2,
                        start=(ib == 1),
                        stop=(ib == NB - 1),
                        perf_mode=mybir.MatmulPerfMode.DoubleRow,
                        skip_group_check=True,
                    )

            # unnormalized ctx -> SBUF bf16 (independent of S: overlaps)
            ctx16 = ctxp.tile([P, EC, 512], BF16, tag="ctx16")
            for ec in range(EC):
                nc.scalar.copy(out=ctx16[:, ec, :], in_=ctx_ps[:, ec, :])

            s_sb = rowp.tile([1, 512], F32, tag="s_sb")
            nc.vector.tensor_copy(out=s_sb, in_=s_ps)

            # out_pre = ctx_unnorm @ Wc ; scale rows by 1/S ; residual + LN
            # pass 1: x = out_pre/S + xres, batched mean/var
            x_tiles = []
            mv4 = epi.tile([P, 4, 2], F32, tag="mv4")
            s_col = None
            rs_col = None
            for os4 in range(4):
                opre = sps.tile([P, 512], F32, tag="sps")
                for ec in range(EC):
                    nc.tensor.matmul(
                        out=opre,
                        lhsT=ctx16[:, ec, os4 * P : (os4 + 1) * P],
                        rhs=wc16[:, ec, :],
                        start=(ec == 0),
                        stop=(ec == EC - 1),
                        skip_group_check=True,
                    )
                if os4 == 0:
                    # 1/S per-partition: S row -> PE transpose -> recip;
                    # emitted after the first Wc group so PE never waits
                    # on the DVE S-copy.
                    s_col = sps.tile([P, 4], F32, tag="sps")
                    for j in range(4):
                        nc.tensor.matmul(
                            out=s_col[:, j : j + 1],
                            lhsT=s_sb[0:1, j * P : (j + 1) * P],
                            rhs=one32,
                            is_transpose=True,
                            start=(j == 0),
                            stop=(j == 3),
                            skip_group_check=True,
                        )
                    rs_col = rowp.tile([P, 4], F32, tag="rs_col")
                    nc.vector.reciprocal(out=rs_col, in_=s_col)
                t0 = epi.tile([P, D], F32, tag="t0")
                nc.scalar.mul(t0, opre, rs_col[:, os4 : os4 + 1])
                x_sb = xsbp.tile([P, D], F32, tag="x_sb")
                nc.vector.tensor_add(x_sb, t0, xres_tiles[os4])
                x_tiles.append(x_sb)
                stats = epi.tile([P, 6], F32, tag="stats")
                nc.vector.bn_stats(out=stats, in_=x_sb)
                nc.vector.bn_aggr(out=mv4[:, os4, :], in_=stats)
            # one Sqrt + one reciprocal per o-chunk (avoids ACT table thrash)
            sd4 = epi.tile([P, 4], F32, tag="sd4")
            nc.scalar.activation(
                out=sd4,
                in_=mv4[:, :, 1],
                func=mybir.ActivationFunctionType.Sqrt,
                bias=eps_t[:, 0:1],
                scale=1.0,
            )
            rs4 = epi.tile([P, 4], F32, tag="rs4")
            nc.vector.reciprocal(out=rs4, in_=sd4)
            # pass 2: normalize (DVE), gamma (gpsimd), beta (DVE)
            for os4 in range(4):
                t_sb = epi.tile([P, D], F32, tag="t_sb")
                nc.vector.tensor_scalar(
                    t_sb,
                    x_tiles[os4],
                    mv4[:, os4, 0:1],
                    rs4[:, os4 : os4 + 1],
                    mybir.AluOpType.subtract,
                    mybir.AluOpType.mult,
                )
                g_sb = epi.tile([P, D], F32, tag="g_sb")
                nc.gpsimd.tensor_mul(g_sb, t_sb, gamma_b)
                o_sb = epi.tile([P, D], F32, tag="o_sb")
                nc.vector.tensor_add(o_sb, g_sb, beta_b)
                r0 = (oc * 4 + os4) * P
                nc.sync.dma_start(out=out_ext[r0 : r0 + P, :], in_=o_sb)

    _split_excess_waits(nc)
    return nc


_NC_CACHE = None


def kernel(**inputs) -> np.ndarray:
    global _NC_CACHE
    _apply_patches()
    from concourse.bass_utils import run_bass_kernel_spmd

    node_fts = np.ascontiguousarray(np.asarray(inputs["node_fts"], dtype=np.float32))
    rel_edges = np.ascontiguousarray(np.asarray(inputs["rel_edges"], dtype=np.float32))
    shared = {
        k: np.ascontiguousarray(np.asarray(inputs[k], dtype=np.float32))
        for k in ("Wq", "bq", "Wk", "bk", "Wc", "gamma", "beta")
    }
    if _NC_CACHE is None:
        _NC_CACHE = build_nc()
    in_maps = [
        {"node_fts": node_fts[b], "rel_edges": rel_edges[b], **shared}
        for b in range(B)
    ]
    res = run_bass_kernel_spmd(_NC_CACHE, in_maps, core_ids=list(range(B)))
    return np.stack([res.results[b]["out"] for b in range(B)]).astype(np.float32)
